# revision 33
# baseline (speedup 1.0000x reference)
"""Trainium2 Bass kernel for BoundaryFeaturePropagation.

Sharding: data-parallel over batch — one batch image per NeuronCore
(B=8 over 8 cores); the small [C,C] weights are replicated on all cores.

Per-core pipeline:
  1. gate:   conf = clip(1 - beta*sigmoid(a*sigmoid(bl) - g), 0, 1), plus
             PE transposes to build per-step gate rows for all 4 directions.
  2. scan:   4 directional gated RNN scans run concurrently, state layout
             [c(part), n(free)].  Per step: input-proj matmuls (batched in
             2-step PSUM blocks, bf16), state matmuls accumulate into the
             same PSUM bank, DVE scalar_tensor_tensor computes the next
             gated state hg = relu(psum)*g directly from PSUM (relu/gate
             commute since g>=0), ACT evacs h = relu(psum) into a history
             ring, and periodic block adds accumulate the ring into the
             fused accumulator.
  3. output: fused -> out_w projection (two passes: one for GroupNorm
             stats, one for normalized output), residual add from the
             bf16-resident input, GN affine folded into one ACT op.
All matmuls are bf16 with fp32 PSUM accumulation.
"""

import os
import sys

for _p in ("/opt/trn_rl_repo",):
    if _p not in sys.path and os.path.isdir(_p):
        sys.path.insert(0, _p)

import numpy as np
import ml_dtypes
from contextlib import ExitStack

import concourse.bass as bass
import concourse.bacc as bacc
import concourse.mybir as mybir
import concourse.tile as tile
from concourse import library_config
from concourse.bass_utils import run_bass_kernel_spmd

BF = ml_dtypes.bfloat16
F32 = mybir.dt.float32
DBF = mybir.dt.bfloat16
AF = mybir.ActivationFunctionType
OP = mybir.AluOpType

ALPHA = 20.0
GAMMA = 4.0
GN_GROUPS = 32
EPS = 1e-5

C = 256
NK = 2          # c-tiles (k/m halves of 128)
S = 128         # H = W
HW = S * S
DEPTH = 8       # history ring slots per direction
ABLK = 4        # steps per acc block-add
PBLK = 2        # steps per psum proj block
CH = 512        # output-phase chunk (positions)
NCH = HW // CH
HG_FROM_HIST = os.environ.get("K_HG_FROM_HIST", "0") == "1"


def _mkap(t, off, dims):
    """Custom free-dim AP on a tile: dims = [[step, count], ...] (outer->inner),
    off in elements of the tile's free space."""
    a = t[:]
    return bass.AP(a.tensor, a.offset + off, [list(a.ap[0])] + [list(d) for d in dims])


def _dram_ap(d, off, dims):
    a = d[:] if not isinstance(d, bass.AP) else d
    return bass.AP(a.tensor, off, [list(x) for x in dims])


def build_program(beta, use_kbias):
    """Build the SPMD single-core program (same on all 8 cores)."""
    nc = bacc.Bacc("TRN2", target_bir_lowering=False, debug=False)

    # ---- DRAM I/O ----
    xb_d = nc.dram_tensor("xb", [C, S, S], DBF, kind="ExternalInput")
    bl_d = nc.dram_tensor("bl", [S, S], F32, kind="ExternalInput")
    wi_d = nc.dram_tensor("wi_t", [4, C, C], DBF, kind="ExternalInput")
    ws_d = nc.dram_tensor("ws_t", [4, C, C], DBF, kind="ExternalInput")
    wo_d = nc.dram_tensor("wo_t", [C, C], DBF, kind="ExternalInput")
    ob_d = nc.dram_tensor("ob", [C], F32, kind="ExternalInput")
    gnw_d = nc.dram_tensor("gnw", [C], F32, kind="ExternalInput")
    gnb_d = nc.dram_tensor("gnb", [C], F32, kind="ExternalInput")
    gsel_d = nc.dram_tensor("gsel", [2, 128, GN_GROUPS], F32, kind="ExternalInput")
    gexp_d = nc.dram_tensor("gexp", [2, GN_GROUPS, 128], F32, kind="ExternalInput")
    id_d = nc.dram_tensor("ident", [128, 128], F32, kind="ExternalInput")
    rid_d = nc.dram_tensor("revid", [128, 128], F32, kind="ExternalInput")
    if use_kbias:
        kb_d = nc.dram_tensor("kb", [4, C], DBF, kind="ExternalInput")
    gd = nc.dram_tensor("gdram", [S, 4, S], DBF)
    out_d = nc.dram_tensor("out", [C, S, S], F32, kind="ExternalOutput")

    with tile.TileContext(nc) as tc:
        with ExitStack() as ctx:
            cp = ctx.enter_context(tc.tile_pool(name="const", bufs=1))

            # ---- resident tensors ----
            x = [cp.tile([128, HW], DBF, tag=f"x{k}", name=f"x{k}") for k in range(NK)]
            acc = [cp.tile([128, HW], DBF, tag=f"acc{k}", name=f"acc{k}") for k in range(NK)]
            hist = [cp.tile([128, 4, DEPTH, S], DBF, tag=f"hist{k}", name=f"hist{k}") for k in range(NK)]
            wi = [cp.tile([128, 4, NK, 128], DBF, tag=f"wi{k}", name=f"wi{k}") for k in range(NK)]
            ws = [cp.tile([128, 4, NK, 128], DBF, tag=f"ws{k}", name=f"ws{k}") for k in range(NK)]
            wo = [cp.tile([128, NK, 128], DBF, tag=f"wo{k}", name=f"wo{k}") for k in range(NK)]
            G = cp.tile([128, 4, S], DBF, tag="G")            # gate rows per step t
            bl = cp.tile([128, S], F32, tag="bl")
            conf = cp.tile([128, S], F32, tag="conf")
            confT = cp.tile([128, S], F32, tag="confT")
            confTr = cp.tile([128, S], F32, tag="confTr")
            confr = cp.tile([128, S], F32, tag="confr")
            ident = cp.tile([128, 128], F32, tag="ident")
            revid = cp.tile([128, 128], F32, tag="revid")
            ob = [cp.tile([128, 1], F32, tag=f"ob{k}", name=f"ob{k}") for k in range(NK)]
            gnw = [cp.tile([128, 1], F32, tag=f"gnw{k}", name=f"gnw{k}") for k in range(NK)]
            gnb = [cp.tile([128, 1], F32, tag=f"gnb{k}", name=f"gnb{k}") for k in range(NK)]
            gsel = [cp.tile([128, GN_GROUPS], F32, tag=f"gsel{k}", name=f"gsel{k}") for k in range(NK)]
            gexp = [cp.tile([GN_GROUPS, 128], F32, tag=f"gexp{k}", name=f"gexp{k}") for k in range(NK)]
            sums = [cp.tile([128, NCH], F32, tag=f"sums{k}", name=f"sums{k}") for k in range(NK)]
            sumsq = [cp.tile([128, NCH], F32, tag=f"sumsq{k}", name=f"sumsq{k}") for k in range(NK)]
            if use_kbias:
                kb = cp.tile([1, 4 * C], DBF, tag="kb")
                ones_row = cp.tile([1, PBLK * S], DBF, tag="ones_row")
                nc.vector.memset(ones_row[:], 1.0)

            # ---- DMAs in ----
            NXC = 8
            for k in range(NK):
                for j in range(NXC):
                    sz = HW // NXC
                    nc.sync.dma_start(
                        x[k][:, j * sz:(j + 1) * sz],
                        _dram_ap(xb_d, k * 128 * HW + j * sz,
                                 [[HW, 128], [1, sz]]))
                for i in range(4):
                    nc.sync.dma_start(
                        wi[k][:, i, :, :],
                        _dram_ap(wi_d, i * C * C + k * 128 * C,
                                 [[C, 128], [128, NK], [1, 128]]))
                    nc.sync.dma_start(
                        ws[k][:, i, :, :],
                        _dram_ap(ws_d, i * C * C + k * 128 * C,
                                 [[C, 128], [128, NK], [1, 128]]))
                nc.sync.dma_start(
                    wo[k][:], _dram_ap(wo_d, k * 128 * C,
                                       [[C, 128], [128, NK], [1, 128]]))
                nc.sync.dma_start(ob[k][:], _dram_ap(ob_d, k * 128, [[1, 128], [1, 1]]))
                nc.sync.dma_start(gnw[k][:], _dram_ap(gnw_d, k * 128, [[1, 128], [1, 1]]))
                nc.sync.dma_start(gnb[k][:], _dram_ap(gnb_d, k * 128, [[1, 128], [1, 1]]))
                nc.sync.dma_start(gsel[k][:], _dram_ap(gsel_d, k * 128 * GN_GROUPS,
                                                       [[GN_GROUPS, 128], [1, GN_GROUPS]]))
                nc.sync.dma_start(gexp[k][:], _dram_ap(gexp_d, k * GN_GROUPS * 128,
                                                       [[128, GN_GROUPS], [1, 128]]))
            with tc.high_priority():
                nc.sync.dma_start(bl[:, :], _dram_ap(bl_d, 0, [[S, 128], [1, S]]))
                nc.sync.dma_start(ident[:], id_d[:])
                nc.sync.dma_start(revid[:], rid_d[:])
            if use_kbias:
                nc.sync.dma_start(kb[:], _dram_ap(kb_d, 0, [[4 * C, 1], [1, 4 * C]]))

            # ---- gate computation (prioritized: it gates the scan) ----
            ctx_g = tc.high_priority()
            ctx_g.__enter__()
            s1 = cp.tile([128, S], F32, tag="s1")
            ngam = cp.tile([128, 1], F32, tag="ngam")
            nc.vector.memset(ngam[:], -GAMMA)
            epsb = cp.tile([GN_GROUPS, 1], F32, tag="epsb")
            nc.vector.memset(epsb[:], EPS)
            nc.scalar.activation(s1[:], bl[:], AF.Sigmoid)
            nc.scalar.activation(conf[:], s1[:], AF.Sigmoid, bias=ngam[:, 0:1],
                                 scale=ALPHA)
            # conf = clip(1 - beta*s2, 0, 1)
            nc.vector.tensor_scalar(conf[:], conf[:], -float(beta), 1.0, OP.mult, OP.add)
            nc.vector.tensor_scalar(conf[:], conf[:], 0.0, 1.0, OP.max, OP.min)

            # transposes: confT[t,n]=conf[n,t]; partition reversals via the
            # anti-diagonal permutation: rev(M) = revid.T @ M.
            # confr[t,n]=conf[S-1-t,n]; confTr[t,n]=conf[n,S-1-t]=confT[S-1-t,n]
            with tc.tile_pool(name="tp_ps", bufs=1, space="PSUM") as tps:
                pt = tps.tile([128, 128], F32, tag="pt")
                nc.tensor.transpose(pt[:], conf[:], ident[:])
                nc.vector.tensor_copy(confT[:], pt[:])
                pt2 = tps.tile([128, 128], F32, tag="pt2")
                nc.tensor.matmul(pt2[:], revid[:], conf[:], start=True, stop=True)
                nc.vector.tensor_copy(confr[:], pt2[:])
                pt3 = tps.tile([128, 128], F32, tag="pt3")
                nc.tensor.matmul(pt3[:], revid[:], confT[:], start=True, stop=True)
                nc.vector.tensor_copy(confTr[:], pt3[:])

            # assemble G[t, dir, n] (bf16): lr=confT, rl=confTr, tb=conf, bt=confr
            nc.vector.tensor_copy(G[:, 0, :], confT[:])
            nc.vector.tensor_copy(G[:, 1, :], confTr[:])
            nc.vector.tensor_copy(G[:, 2, :], conf[:])
            nc.vector.tensor_copy(G[:, 3, :], confr[:])
            # stage gate table to DRAM for per-step stride-0 broadcast reads
            nc.sync.dma_start(_dram_ap(gd, 0, [[4 * S, 128], [1, 4 * S]]), G[:])
            ctx_g.__exit__(None, None, None)

            for k in range(NK):
                nc.gpsimd.memset(acc[k][:], 0.0)

            # ================= SCAN =================
            # direction d: 0=lr, 1=rl, 2=tb, 3=bt
            # lr/tb get one PBLK-step proj matmul; rl/bt (reversed walks)
            # get per-step matmuls so all APs keep positive strides.
            def proj_rhs_blk(k, d, t0):
                if d == 0:
                    return _mkap(x[k], t0, [[1, PBLK], [S, S]])
                return _mkap(x[k], t0 * S, [[S, PBLK], [1, S]])

            def proj_rhs_step(k, d, t):
                if d == 1:
                    return _mkap(x[k], S - 1 - t, [[S, S]])
                return _mkap(x[k], (S - 1 - t) * S, [[1, S]])

            # zero-region (2KB) boundaries inside the [128, 4, PBLK*S] psum tile:
            dir_bytes = PBLK * S * 4
            first_in_zr = [d for d in range(4) if (d * dir_bytes) % 2048 == 0]
            last_in_zr = [d for d in range(4)
                          if ((d + 1) * dir_bytes) % 2048 == 0 or d == 3]

            gbp = ctx.enter_context(tc.tile_pool(name="gb", bufs=int(os.environ.get("K_GB", "6"))))
            hgp = ctx.enter_context(tc.tile_pool(name="hg", bufs=int(os.environ.get("K_HG", "4"))))
            with ExitStack() as sctx:
                psp = [sctx.enter_context(
                        tc.tile_pool(name=f"ps{m}", bufs=2, space="PSUM"))
                       for m in range(NK)]
                nblocks = S // PBLK
                hg_cur = None
                ps_tiles = {}

                def alloc_ps(b):
                    if b not in ps_tiles and b < nblocks:
                        ps_tiles[b] = [psp[m].tile([128, 4, PBLK * S], F32,
                                                   tag=f"psb{m}", name=f"psb{m}")
                                       for m in range(NK)]
                    return ps_tiles.get(b)

                def emit_proj(b, m):
                    """Input-projection matmuls for block b, output half m."""
                    if b >= nblocks:
                        return
                    t0 = PBLK * b
                    ps = ps_tiles[b]
                    for d in range(4):
                        for k in range(NK):
                            if d in (0, 2):
                                nc.tensor.matmul(
                                    ps[m][:, d, :],
                                    wi[k][:, d, m, :],
                                    proj_rhs_blk(k, d, t0),
                                    start=(k == 0 and d in (0, 2)),
                                    stop=False, skip_group_check=True)
                            else:
                                for ti in range(PBLK):
                                    nc.tensor.matmul(
                                        ps[m][:, d, ti * S:(ti + 1) * S],
                                        wi[k][:, d, m, :],
                                        proj_rhs_step(k, d, t0 + ti),
                                        start=False,
                                        stop=False, skip_group_check=True)
                        if use_kbias:
                            nc.tensor.matmul(
                                ps[m][:, d, :],
                                kb[:, d * C + m * 128: d * C + (m + 1) * 128],
                                ones_row[:],
                                start=False, stop=False, skip_group_check=True)

                # prefetch ALL per-step gate rows up front; the pool's
                # slot reuse (bufs) self-paces the DMAs against consumers
                gb_tiles = {}
                for tt in range(1, S):
                    g_t = gbp.tile([128, 4, S], DBF, tag="gb", name="gb")
                    nc.sync.dma_start(
                        g_t[:], _dram_ap(gd, tt * 4 * S,
                                         [[0, 128], [1, 4 * S]]))
                    gb_tiles[tt] = g_t

                alloc_ps(0)
                for m in range(NK):
                    emit_proj(0, m)
                for b in range(nblocks):
                    t0 = PBLK * b
                    ps = ps_tiles.pop(b)
                    alloc_ps(b + 1)
                    for ti in range(PBLK):
                        t = t0 + ti
                        slot = t % DEPTH
                        # --- state matmuls (skip t=0), m-major: ps[0] is
                        # complete after the first half, so the m0 gate op
                        # overlaps the m1 matmuls; next step's k-consumers
                        # of hg[0] come first so hg[1] can still be in flight ---
                        if t + 1 < S:
                            gb = gb_tiles.pop(t + 1)
                            hg_nxt = [hgp.tile([128, 4, S], DBF, tag=f"hg{m}",
                                               name=f"hg{m}")
                                      for m in range(NK)]
                        else:
                            gb = None
                        for m in range(NK):
                            if t > 0:
                                for k in range(NK):
                                    for d in range(4):
                                        nc.tensor.matmul(
                                            ps[m][:, d, ti * S:(ti + 1) * S],
                                            ws[k][:, d, m, :],
                                            hg_cur[k][:, d, :],
                                            start=False,
                                            stop=(ti == PBLK - 1 and k == NK - 1
                                                  and d % 2 == 1),
                                            skip_group_check=True)
                            # gate op for this half fires as soon as its own
                            # matmuls are done (boosted: heads the DVE queue)
                            if gb is not None:
                                with tc.high_priority(offset=int(os.environ.get("K_STTP", "0"))):
                                    nc.vector.scalar_tensor_tensor(
                                        hg_nxt[m][:], ps[m][:, :, ti * S:(ti + 1) * S],
                                        0.0, gb[:], OP.max, OP.mult)
                        if gb is not None:
                            hg_cur = hg_nxt
                        # next block's proj fills the PE gap during the gate op
                        emit_proj(b + 1, ti)
                        # --- history evac: h = relu(psum), both block steps in
                        # one ACT op per m-half ---
                        if ti == PBLK - 1:
                            for m in range(NK):
                                nc.scalar.activation(
                                    hist[m][:, :, slot - 1:slot + 1, :],
                                    ps[m][:, :, :], AF.Relu)
                        # --- acc block adds every ABLK steps (gpsimd;
                        # deprioritized so the gate op wins the queues) ---
                        if t % ABLK == ABLK - 1:
                            tb0 = t - (ABLK - 1)
                            s0 = tb0 % DEPTH
                            ctx_p = tc.high_priority(offset=-int(os.environ.get('K_PRIO', '128')))
                            ctx_p.__enter__()
                            for k in range(NK):
                                # lr: cols tb0..t (h outer, w inner)
                                nc.gpsimd.tensor_tensor(
                                    _mkap(acc[k], tb0, [[S, S], [1, ABLK]]),
                                    _mkap(acc[k], tb0, [[S, S], [1, ABLK]]),
                                    _mkap(hist[k], (0 * DEPTH + s0) * S,
                                          [[1, S], [S, ABLK]]),
                                    OP.add)
                                # rl: cols S-1-tb0 down
                                nc.vector.tensor_tensor(
                                    _mkap(acc[k], S - 1 - tb0, [[S, S], [-1, ABLK]]),
                                    _mkap(acc[k], S - 1 - tb0, [[S, S], [-1, ABLK]]),
                                    _mkap(hist[k], (1 * DEPTH + s0) * S,
                                          [[1, S], [S, ABLK]]),
                                    OP.add)
                                # tb: rows tb0..t (slot outer, w inner)
                                nc.gpsimd.tensor_tensor(
                                    _mkap(acc[k], tb0 * S, [[S, ABLK], [1, S]]),
                                    _mkap(acc[k], tb0 * S, [[S, ABLK], [1, S]]),
                                    hist[k][:, 2, s0:s0 + ABLK, :],
                                    OP.add)
                                # bt: rows S-1-tb0 down
                                nc.vector.tensor_tensor(
                                    _mkap(acc[k], (S - 1 - tb0) * S, [[-S, ABLK], [1, S]]),
                                    _mkap(acc[k], (S - 1 - tb0) * S, [[-S, ABLK], [1, S]]),
                                    hist[k][:, 3, s0:s0 + ABLK, :],
                                    OP.add)
                            ctx_p.__exit__(None, None, None)

            # ================= OUTPUT =================
            inv_n = 1.0 / (8.0 * HW)
            with (
                tc.tile_pool(name="ops", bufs=2, space="PSUM") as ops_pool,
                tc.tile_pool(name="oz", bufs=2) as ozp,
                tc.tile_pool(name="ost", bufs=1) as ostp,
                tc.tile_pool(name="obuf", bufs=2) as obp,
            ):
                # ---- pass 1: stats ----
                for j in range(NCH):
                    for m in range(NK):
                        pso = ops_pool.tile([128, CH], F32, tag=f"pso{m}")
                        for k in range(NK):
                            nc.tensor.matmul(pso[:], wo[k][:, m, :],
                                             acc[k][:, j * CH:(j + 1) * CH],
                                             start=(k == 0), stop=(k == NK - 1))
                        z = ozp.tile([128, CH], F32, tag=f"z{m}")
                        nc.vector.scalar_tensor_tensor(
                            z[:], pso[:], ob[m][:, 0:1], x[m][:, j * CH:(j + 1) * CH],
                            OP.add, OP.add, accum_out=sums[m][:, j:j + 1])
                        junk = ozp.tile([128, CH], F32, tag="junk")
                        nc.scalar.activation(junk[:], z[:], AF.Square,
                                             accum_out=sumsq[m][:, j:j + 1])
                # ---- group stats ----
                ssq = [ostp.tile([128, 2], F32, tag=f"ssq{k}", name=f"ssq{k}") for k in range(NK)]
                for k in range(NK):
                    nc.vector.tensor_reduce(ssq[k][:, 0:1], sums[k][:, 0:NCH],
                                            mybir.AxisListType.X, OP.add)
                    nc.vector.tensor_reduce(ssq[k][:, 1:2], sumsq[k][:, 0:NCH],
                                            mybir.AxisListType.X, OP.add)
                with tc.tile_pool(name="stps", bufs=1, space="PSUM") as stps:
                    psg = stps.tile([GN_GROUPS, 2], F32, tag="psg")
                    for k in range(NK):
                        nc.tensor.matmul(psg[:], gsel[k][:], ssq[k][:],
                                         start=(k == 0), stop=(k == NK - 1))
                    mv = ostp.tile([GN_GROUPS, 2], F32, tag="mv")
                    nc.vector.tensor_scalar(mv[:], psg[:], inv_n, None, OP.mult)
                    mu2 = ostp.tile([GN_GROUPS, 1], F32, tag="mu2")
                    nc.vector.tensor_tensor(mu2[:], mv[:, 0:1], mv[:, 0:1], OP.mult)
                    var = ostp.tile([GN_GROUPS, 1], F32, tag="var")
                    nc.vector.tensor_tensor(var[:], mv[:, 1:2], mu2[:], OP.subtract)
                    sd = ostp.tile([GN_GROUPS, 1], F32, tag="sd")
                    nc.scalar.activation(sd[:], var[:], AF.Sqrt, bias=epsb[:, 0:1])
                    rstd = ostp.tile([GN_GROUPS, 1], F32, tag="rstd")
                    nc.vector.reciprocal(rstd[:], sd[:])
                    mr = ostp.tile([GN_GROUPS, 2], F32, tag="mr")
                    nc.vector.tensor_copy(mr[:, 0:1], mv[:, 0:1])
                    nc.vector.tensor_copy(mr[:, 1:2], rstd[:])
                    # expand group stats to channels; fold gn affine
                    scale = [ostp.tile([128, 1], F32, tag=f"scale{k}", name=f"scale{k}") for k in range(NK)]
                    bias = [ostp.tile([128, 1], F32, tag=f"bias{k}", name=f"bias{k}") for k in range(NK)]
                    for k in range(NK):
                        pse = stps.tile([128, 2], F32, tag=f"pse{k}")
                        nc.tensor.matmul(pse[:], gexp[k][:], mr[:], start=True, stop=True)
                        muc = ostp.tile([128, 1], F32, tag=f"muc{k}")
                        rc = ostp.tile([128, 1], F32, tag=f"rc{k}")
                        nc.vector.tensor_copy(muc[:], pse[:, 0:1])
                        nc.vector.tensor_copy(rc[:], pse[:, 1:2])
                        nc.vector.tensor_tensor(scale[k][:], rc[:], gnw[k][:], OP.mult)
                        tmp = ostp.tile([128, 1], F32, tag=f"tmp{k}")
                        nc.vector.tensor_tensor(tmp[:], muc[:], scale[k][:], OP.mult)
                        nc.vector.tensor_tensor(bias[k][:], gnb[k][:], tmp[:], OP.subtract)

                    # ---- pass 2: normalized output ----
                    for j in range(NCH):
                        for m in range(NK):
                            pso = ops_pool.tile([128, CH], F32, tag=f"pso{m}")
                            for k in range(NK):
                                nc.tensor.matmul(pso[:], wo[k][:, m, :],
                                                 acc[k][:, j * CH:(j + 1) * CH],
                                                 start=(k == 0), stop=(k == NK - 1))
                            z = ozp.tile([128, CH], F32, tag=f"z{m}")
                            nc.vector.scalar_tensor_tensor(
                                z[:], pso[:], ob[m][:, 0:1],
                                x[m][:, j * CH:(j + 1) * CH], OP.add, OP.add)
                            of = obp.tile([128, CH], F32, tag=f"of{m}",
                                          name=f"of{m}")
                            nc.scalar.activation(of[:], z[:], AF.Identity,
                                                 bias=bias[m][:, 0:1],
                                                 scale=scale[m][:, 0:1])
                            nc.sync.dma_start(
                                _dram_ap(out_d, m * 128 * HW + j * CH,
                                         [[HW, 128], [1, CH]]),
                                of[:])
    nc.compile()
    return nc


_CACHE = {}


def _get_program(beta, use_kbias):
    key = (float(beta), bool(use_kbias))
    if key not in _CACHE:
        _CACHE[key] = build_program(beta, use_kbias)
    return _CACHE[key]


def make_host_inputs(feature, boundary_logits, beta, W_in, b_in, W_s, b_s,
                     p_bias, out_w, out_b, gn_w, gn_b):
    wi_t = np.ascontiguousarray(
        np.transpose(np.asarray(W_in, np.float32), (0, 2, 1))).astype(BF)
    ws_t = np.ascontiguousarray(
        np.transpose(np.asarray(W_s, np.float32), (0, 2, 1))).astype(BF)
    wo_t = np.ascontiguousarray(np.asarray(out_w, np.float32).T).astype(BF)
    kbv = (np.asarray(b_in, np.float32) + np.asarray(b_s, np.float32)
           + np.asarray(p_bias, np.float32))
    use_kbias = bool(np.any(kbv != 0.0))
    cpg = C // GN_GROUPS
    gsel = np.zeros((2, 128, GN_GROUPS), np.float32)
    gexp = np.zeros((2, GN_GROUPS, 128), np.float32)
    for k in range(2):
        for p in range(128):
            g = (k * 128 + p) // cpg
            gsel[k, p, g] = 1.0
            gexp[k, g, p] = 1.0
    common = {
        "wi_t": wi_t, "ws_t": ws_t, "wo_t": wo_t,
        "ob": np.asarray(out_b, np.float32),
        "gnw": np.asarray(gn_w, np.float32),
        "gnb": np.asarray(gn_b, np.float32),
        "gsel": gsel, "gexp": gexp,
        "ident": np.eye(128, dtype=np.float32),
        "revid": np.eye(128, dtype=np.float32)[::-1].copy(),
    }
    if use_kbias:
        common["kb"] = kbv.astype(BF)
    B = np.asarray(feature).shape[0]
    in_maps = []
    for b in range(B):
        m = dict(common)
        m["xb"] = np.asarray(feature[b], np.float32).astype(BF)
        m["bl"] = np.asarray(boundary_logits[b], np.float32).reshape(S, S)
        in_maps.append(m)
    return in_maps, float(np.asarray(beta).reshape(-1)[0]), use_kbias


def kernel(feature, boundary_logits, beta, W_in, b_in, W_s, b_s, p_bias,
           out_w, out_b, gn_w, gn_b):
    feature = np.asarray(feature)
    B = feature.shape[0]
    in_maps, beta_v, use_kbias = make_host_inputs(
        feature, boundary_logits, beta, W_in, b_in, W_s, b_s, p_bias,
        out_w, out_b, gn_w, gn_b)
    nc = _get_program(beta_v, use_kbias)
    res = run_bass_kernel_spmd(nc, in_maps, core_ids=list(range(B)))
    out = np.stack([np.asarray(r["out"]) for r in res.results], axis=0)
    return out.astype(np.float32)



# revision 37
# speedup vs baseline: 1.0262x; 1.0262x over previous
"""Trainium2 Bass kernel for BoundaryFeaturePropagation.

Sharding: data-parallel over batch — one batch image per NeuronCore
(B=8 over 8 cores); the small [C,C] weights are replicated on all cores.

Per-core pipeline:
  1. gate:   conf = clip(1 - beta*sigmoid(a*sigmoid(bl) - g), 0, 1), plus
             PE transposes to build per-step gate rows for all 4 directions.
  2. scan:   4 directional gated RNN scans run concurrently, state layout
             [c(part), n(free)].  Per step: input-proj matmuls (batched in
             2-step PSUM blocks, bf16), state matmuls accumulate into the
             same PSUM bank, DVE scalar_tensor_tensor computes the next
             gated state hg = relu(psum)*g directly from PSUM (relu/gate
             commute since g>=0), ACT evacs h = relu(psum) into a history
             ring, and periodic block adds accumulate the ring into the
             fused accumulator.
  3. output: fused -> out_w projection (two passes: one for GroupNorm
             stats, one for normalized output), residual add from the
             bf16-resident input, GN affine folded into one ACT op.
All matmuls are bf16 with fp32 PSUM accumulation.
"""

import os
import sys

for _p in ("/opt/trn_rl_repo",):
    if _p not in sys.path and os.path.isdir(_p):
        sys.path.insert(0, _p)

import numpy as np
import ml_dtypes
from contextlib import ExitStack

import concourse.bass as bass
import concourse.bacc as bacc
import concourse.mybir as mybir
import concourse.tile as tile
from concourse import library_config
from concourse.bass_utils import run_bass_kernel_spmd

BF = ml_dtypes.bfloat16
F32 = mybir.dt.float32
DBF = mybir.dt.bfloat16
AF = mybir.ActivationFunctionType
OP = mybir.AluOpType

ALPHA = 20.0
GAMMA = 4.0
GN_GROUPS = 32
EPS = 1e-5

C = 256
NK = 2          # c-tiles (k/m halves of 128)
S = 128         # H = W
HW = S * S
DEPTH = 8       # history ring slots per direction
ABLK = 4        # steps per acc block-add
PBLK = 2        # steps per psum proj block
CH = 512        # output-phase chunk (positions)
NCH = HW // CH
HG_FROM_HIST = os.environ.get("K_HG_FROM_HIST", "0") == "1"


def _mkap(t, off, dims):
    """Custom free-dim AP on a tile: dims = [[step, count], ...] (outer->inner),
    off in elements of the tile's free space."""
    a = t[:]
    return bass.AP(a.tensor, a.offset + off, [list(a.ap[0])] + [list(d) for d in dims])


def _dram_ap(d, off, dims):
    a = d[:] if not isinstance(d, bass.AP) else d
    return bass.AP(a.tensor, off, [list(x) for x in dims])


def build_program(beta, use_kbias):
    """Build the SPMD single-core program (same on all 8 cores)."""
    nc = bacc.Bacc("TRN2", target_bir_lowering=False, debug=False)

    # ---- DRAM I/O ----
    xb_d = nc.dram_tensor("xb", [C, S, S], DBF, kind="ExternalInput")
    bl_d = nc.dram_tensor("bl", [S, S], F32, kind="ExternalInput")
    wi_d = nc.dram_tensor("wi_t", [4, C, C], DBF, kind="ExternalInput")
    ws_d = nc.dram_tensor("ws_t", [4, C, C], DBF, kind="ExternalInput")
    wo_d = nc.dram_tensor("wo_t", [C, C], DBF, kind="ExternalInput")
    ob_d = nc.dram_tensor("ob", [C], F32, kind="ExternalInput")
    gnw_d = nc.dram_tensor("gnw", [C], F32, kind="ExternalInput")
    gnb_d = nc.dram_tensor("gnb", [C], F32, kind="ExternalInput")
    gsel_d = nc.dram_tensor("gsel", [2, 128, GN_GROUPS], F32, kind="ExternalInput")
    gexp_d = nc.dram_tensor("gexp", [2, GN_GROUPS, 128], F32, kind="ExternalInput")
    id_d = nc.dram_tensor("ident", [128, 128], F32, kind="ExternalInput")
    rid_d = nc.dram_tensor("revid", [128, 128], F32, kind="ExternalInput")
    if use_kbias:
        kb_d = nc.dram_tensor("kb", [4, C], DBF, kind="ExternalInput")
    gd = nc.dram_tensor("gdram", [S, 4, S], DBF)
    out_d = nc.dram_tensor("out", [C, S, S], DBF, kind="ExternalOutput")

    with tile.TileContext(nc) as tc:
        with ExitStack() as ctx:
            cp = ctx.enter_context(tc.tile_pool(name="const", bufs=1))

            # ---- resident tensors ----
            x = [cp.tile([128, HW], DBF, tag=f"x{k}", name=f"x{k}") for k in range(NK)]
            acc = [cp.tile([128, HW], DBF, tag=f"acc{k}", name=f"acc{k}") for k in range(NK)]
            hist = [cp.tile([128, 4, DEPTH, S], DBF, tag=f"hist{k}", name=f"hist{k}") for k in range(NK)]
            wi = [cp.tile([128, 4, NK, 128], DBF, tag=f"wi{k}", name=f"wi{k}") for k in range(NK)]
            ws = [cp.tile([128, 4, NK, 128], DBF, tag=f"ws{k}", name=f"ws{k}") for k in range(NK)]
            wo = [cp.tile([128, NK, 128], DBF, tag=f"wo{k}", name=f"wo{k}") for k in range(NK)]
            G = cp.tile([128, 4, S], DBF, tag="G")            # gate rows per step t
            bl = cp.tile([128, S], F32, tag="bl")
            conf = cp.tile([128, S], F32, tag="conf")
            confT = cp.tile([128, S], F32, tag="confT")
            confTr = cp.tile([128, S], F32, tag="confTr")
            confr = cp.tile([128, S], F32, tag="confr")
            ident = cp.tile([128, 128], F32, tag="ident")
            revid = cp.tile([128, 128], F32, tag="revid")
            ob = [cp.tile([128, 1], F32, tag=f"ob{k}", name=f"ob{k}") for k in range(NK)]
            gnw = [cp.tile([128, 1], F32, tag=f"gnw{k}", name=f"gnw{k}") for k in range(NK)]
            gnb = [cp.tile([128, 1], F32, tag=f"gnb{k}", name=f"gnb{k}") for k in range(NK)]
            gsel = [cp.tile([128, GN_GROUPS], F32, tag=f"gsel{k}", name=f"gsel{k}") for k in range(NK)]
            gexp = [cp.tile([GN_GROUPS, 128], F32, tag=f"gexp{k}", name=f"gexp{k}") for k in range(NK)]
            sums = [cp.tile([128, NCH], F32, tag=f"sums{k}", name=f"sums{k}") for k in range(NK)]
            sumsq = [cp.tile([128, NCH], F32, tag=f"sumsq{k}", name=f"sumsq{k}") for k in range(NK)]
            if use_kbias:
                kb = cp.tile([1, 4 * C], DBF, tag="kb")
                ones_row = cp.tile([1, PBLK * S], DBF, tag="ones_row")
                nc.vector.memset(ones_row[:], 1.0)

            # ---- DMAs in ----
            NXC = 8
            for k in range(NK):
                for j in range(NXC):
                    sz = HW // NXC
                    nc.sync.dma_start(
                        x[k][:, j * sz:(j + 1) * sz],
                        _dram_ap(xb_d, k * 128 * HW + j * sz,
                                 [[HW, 128], [1, sz]]))
                for i in range(4):
                    nc.sync.dma_start(
                        wi[k][:, i, :, :],
                        _dram_ap(wi_d, i * C * C + k * 128 * C,
                                 [[C, 128], [128, NK], [1, 128]]))
                    nc.sync.dma_start(
                        ws[k][:, i, :, :],
                        _dram_ap(ws_d, i * C * C + k * 128 * C,
                                 [[C, 128], [128, NK], [1, 128]]))
                nc.sync.dma_start(
                    wo[k][:], _dram_ap(wo_d, k * 128 * C,
                                       [[C, 128], [128, NK], [1, 128]]))
                nc.sync.dma_start(ob[k][:], _dram_ap(ob_d, k * 128, [[1, 128], [1, 1]]))
                nc.sync.dma_start(gnw[k][:], _dram_ap(gnw_d, k * 128, [[1, 128], [1, 1]]))
                nc.sync.dma_start(gnb[k][:], _dram_ap(gnb_d, k * 128, [[1, 128], [1, 1]]))
                nc.sync.dma_start(gsel[k][:], _dram_ap(gsel_d, k * 128 * GN_GROUPS,
                                                       [[GN_GROUPS, 128], [1, GN_GROUPS]]))
                nc.sync.dma_start(gexp[k][:], _dram_ap(gexp_d, k * GN_GROUPS * 128,
                                                       [[128, GN_GROUPS], [1, 128]]))
            with tc.high_priority():
                nc.sync.dma_start(bl[:, :], _dram_ap(bl_d, 0, [[S, 128], [1, S]]))
                nc.sync.dma_start(ident[:], id_d[:])
                nc.sync.dma_start(revid[:], rid_d[:])
            if use_kbias:
                nc.sync.dma_start(kb[:], _dram_ap(kb_d, 0, [[4 * C, 1], [1, 4 * C]]))

            # ---- gate computation (prioritized: it gates the scan) ----
            ctx_g = tc.high_priority()
            ctx_g.__enter__()
            s1 = cp.tile([128, S], F32, tag="s1")
            ngam = cp.tile([128, 1], F32, tag="ngam")
            nc.vector.memset(ngam[:], -GAMMA)
            epsb = cp.tile([GN_GROUPS, 1], F32, tag="epsb")
            nc.vector.memset(epsb[:], EPS)
            nc.scalar.activation(s1[:], bl[:], AF.Sigmoid)
            nc.scalar.activation(conf[:], s1[:], AF.Sigmoid, bias=ngam[:, 0:1],
                                 scale=ALPHA)
            # conf = clip(1 - beta*s2, 0, 1)
            nc.vector.tensor_scalar(conf[:], conf[:], -float(beta), 1.0, OP.mult, OP.add)
            nc.vector.tensor_scalar(conf[:], conf[:], 0.0, 1.0, OP.max, OP.min)

            # transposes: confT[t,n]=conf[n,t]; partition reversals via the
            # anti-diagonal permutation: rev(M) = revid.T @ M.
            # confr[t,n]=conf[S-1-t,n]; confTr[t,n]=conf[n,S-1-t]=confT[S-1-t,n]
            with tc.tile_pool(name="tp_ps", bufs=1, space="PSUM") as tps:
                pt = tps.tile([128, 128], F32, tag="pt")
                nc.tensor.transpose(pt[:], conf[:], ident[:])
                nc.vector.tensor_copy(confT[:], pt[:])
                pt2 = tps.tile([128, 128], F32, tag="pt2")
                nc.tensor.matmul(pt2[:], revid[:], conf[:], start=True, stop=True)
                nc.vector.tensor_copy(confr[:], pt2[:])
                pt3 = tps.tile([128, 128], F32, tag="pt3")
                nc.tensor.matmul(pt3[:], revid[:], confT[:], start=True, stop=True)
                nc.vector.tensor_copy(confTr[:], pt3[:])

            # assemble G[t, dir, n] (bf16): lr=confT, rl=confTr, tb=conf, bt=confr
            nc.vector.tensor_copy(G[:, 0, :], confT[:])
            nc.vector.tensor_copy(G[:, 1, :], confTr[:])
            nc.vector.tensor_copy(G[:, 2, :], conf[:])
            nc.vector.tensor_copy(G[:, 3, :], confr[:])
            # stage gate table to DRAM for per-step stride-0 broadcast reads
            nc.sync.dma_start(_dram_ap(gd, 0, [[4 * S, 128], [1, 4 * S]]), G[:])
            ctx_g.__exit__(None, None, None)

            ctx_ms = tc.high_priority(offset=-192)
            ctx_ms.__enter__()
            for k in range(NK):
                nc.vector.memset(acc[k][:, 0:HW // 2], 0.0)
                nc.gpsimd.memset(acc[k][:, HW // 2:], 0.0)
            ctx_ms.__exit__(None, None, None)

            # ================= SCAN =================
            # direction d: 0=lr, 1=rl, 2=tb, 3=bt
            # lr/tb get one PBLK-step proj matmul; rl/bt (reversed walks)
            # get per-step matmuls so all APs keep positive strides.
            def proj_rhs_blk(k, d, t0):
                if d == 0:
                    return _mkap(x[k], t0, [[1, PBLK], [S, S]])
                return _mkap(x[k], t0 * S, [[S, PBLK], [1, S]])

            def proj_rhs_step(k, d, t):
                if d == 1:
                    return _mkap(x[k], S - 1 - t, [[S, S]])
                return _mkap(x[k], (S - 1 - t) * S, [[1, S]])

            # zero-region (2KB) boundaries inside the [128, 4, PBLK*S] psum tile:
            dir_bytes = PBLK * S * 4
            first_in_zr = [d for d in range(4) if (d * dir_bytes) % 2048 == 0]
            last_in_zr = [d for d in range(4)
                          if ((d + 1) * dir_bytes) % 2048 == 0 or d == 3]

            gbp = ctx.enter_context(tc.tile_pool(name="gb", bufs=int(os.environ.get("K_GB", "6"))))
            hgp = ctx.enter_context(tc.tile_pool(name="hg", bufs=int(os.environ.get("K_HG", "4"))))
            with ExitStack() as sctx:
                psp = [sctx.enter_context(
                        tc.tile_pool(name=f"ps{m}", bufs=2, space="PSUM"))
                       for m in range(NK)]
                nblocks = S // PBLK
                hg_cur = None
                ps_tiles = {}

                def alloc_ps(b):
                    if b not in ps_tiles and b < nblocks:
                        ps_tiles[b] = [psp[m].tile([128, 4, PBLK * S], F32,
                                                   tag=f"psb{m}", name=f"psb{m}")
                                       for m in range(NK)]
                    return ps_tiles.get(b)

                def emit_proj(b, m):
                    """Input-projection matmuls for block b, output half m."""
                    if b >= nblocks:
                        return
                    t0 = PBLK * b
                    ps = ps_tiles[b]
                    for d in range(4):
                        for k in range(NK):
                            if d in (0, 2):
                                nc.tensor.matmul(
                                    ps[m][:, d, :],
                                    wi[k][:, d, m, :],
                                    proj_rhs_blk(k, d, t0),
                                    start=(k == 0 and d in (0, 2)),
                                    stop=False, skip_group_check=True)
                            else:
                                for ti in range(PBLK):
                                    nc.tensor.matmul(
                                        ps[m][:, d, ti * S:(ti + 1) * S],
                                        wi[k][:, d, m, :],
                                        proj_rhs_step(k, d, t0 + ti),
                                        start=False,
                                        stop=False, skip_group_check=True)
                        if use_kbias:
                            nc.tensor.matmul(
                                ps[m][:, d, :],
                                kb[:, d * C + m * 128: d * C + (m + 1) * 128],
                                ones_row[:],
                                start=False, stop=False, skip_group_check=True)

                # prefetch ALL per-step gate rows up front; the pool's
                # slot reuse (bufs) self-paces the DMAs against consumers
                gb_tiles = {}
                for tt in range(1, S):
                    g_t = gbp.tile([128, 4, S], DBF, tag="gb", name="gb")
                    nc.sync.dma_start(
                        g_t[:], _dram_ap(gd, tt * 4 * S,
                                         [[0, 128], [1, 4 * S]]))
                    gb_tiles[tt] = g_t

                alloc_ps(0)
                for m in range(NK):
                    emit_proj(0, m)
                for b in range(nblocks):
                    t0 = PBLK * b
                    ps = ps_tiles.pop(b)
                    alloc_ps(b + 1)
                    for ti in range(PBLK):
                        t = t0 + ti
                        slot = t % DEPTH
                        # --- state matmuls (skip t=0), m-major: ps[0] is
                        # complete after the first half, so the m0 gate op
                        # overlaps the m1 matmuls; next step's k-consumers
                        # of hg[0] come first so hg[1] can still be in flight ---
                        if t + 1 < S:
                            gb = gb_tiles.pop(t + 1)
                            hg_nxt = [hgp.tile([128, 4, S], DBF, tag=f"hg{m}",
                                               name=f"hg{m}")
                                      for m in range(NK)]
                        else:
                            gb = None
                        for m in range(NK):
                            if t > 0:
                                for k in range(NK):
                                    for d in range(4):
                                        nc.tensor.matmul(
                                            ps[m][:, d, ti * S:(ti + 1) * S],
                                            ws[k][:, d, m, :],
                                            hg_cur[k][:, d, :],
                                            start=False,
                                            stop=(ti == PBLK - 1 and k == NK - 1
                                                  and d % 2 == 1),
                                            skip_group_check=True)
                            # gate op for this half fires as soon as its own
                            # matmuls are done (boosted: heads the DVE queue)
                            if gb is not None:
                                with tc.high_priority(offset=int(os.environ.get("K_STTP", "0"))):
                                    nc.vector.scalar_tensor_tensor(
                                        hg_nxt[m][:], ps[m][:, :, ti * S:(ti + 1) * S],
                                        0.0, gb[:], OP.max, OP.mult)
                        if gb is not None:
                            hg_cur = hg_nxt
                        # next block's proj fills the PE gap during the gate op
                        emit_proj(b + 1, ti)
                        # --- history evac: h = relu(psum), both block steps in
                        # one ACT op per m-half ---
                        if ti == PBLK - 1:
                            for m in range(NK):
                                nc.scalar.activation(
                                    hist[m][:, :, slot - 1:slot + 1, :],
                                    ps[m][:, :, :], AF.Relu)
                        # --- acc block adds every ABLK steps (gpsimd;
                        # deprioritized so the gate op wins the queues) ---
                        if t % ABLK == ABLK - 1:
                            tb0 = t - (ABLK - 1)
                            s0 = tb0 % DEPTH
                            ctx_p = tc.high_priority(offset=-int(os.environ.get('K_PRIO', '128')))
                            ctx_p.__enter__()
                            for k in range(NK):
                                # lr: cols tb0..t (h outer, w inner)
                                nc.gpsimd.tensor_tensor(
                                    _mkap(acc[k], tb0, [[S, S], [1, ABLK]]),
                                    _mkap(acc[k], tb0, [[S, S], [1, ABLK]]),
                                    _mkap(hist[k], (0 * DEPTH + s0) * S,
                                          [[1, S], [S, ABLK]]),
                                    OP.add)
                                # rl: cols S-1-tb0 down
                                nc.vector.tensor_tensor(
                                    _mkap(acc[k], S - 1 - tb0, [[S, S], [-1, ABLK]]),
                                    _mkap(acc[k], S - 1 - tb0, [[S, S], [-1, ABLK]]),
                                    _mkap(hist[k], (1 * DEPTH + s0) * S,
                                          [[1, S], [S, ABLK]]),
                                    OP.add)
                                # tb: rows tb0..t (slot outer, w inner)
                                nc.gpsimd.tensor_tensor(
                                    _mkap(acc[k], tb0 * S, [[S, ABLK], [1, S]]),
                                    _mkap(acc[k], tb0 * S, [[S, ABLK], [1, S]]),
                                    hist[k][:, 2, s0:s0 + ABLK, :],
                                    OP.add)
                                # bt: rows S-1-tb0 down
                                nc.vector.tensor_tensor(
                                    _mkap(acc[k], (S - 1 - tb0) * S, [[-S, ABLK], [1, S]]),
                                    _mkap(acc[k], (S - 1 - tb0) * S, [[-S, ABLK], [1, S]]),
                                    hist[k][:, 3, s0:s0 + ABLK, :],
                                    OP.add)
                            ctx_p.__exit__(None, None, None)

            # ================= OUTPUT =================
            inv_n = 1.0 / (8.0 * HW)
            with (
                tc.tile_pool(name="ops", bufs=2, space="PSUM") as ops_pool,
                tc.tile_pool(name="oz", bufs=2) as ozp,
                tc.tile_pool(name="ost", bufs=1) as ostp,
                tc.tile_pool(name="obuf", bufs=6) as obp,
            ):
                # ---- pass 1: matmuls + z (bf16, in-place over x) + stats ----
                for j in range(NCH):
                    for m in range(NK):
                        pso = ops_pool.tile([128, CH], F32, tag=f"pso{m}")
                        for k in range(NK):
                            nc.tensor.matmul(pso[:], wo[k][:, m, :],
                                             acc[k][:, j * CH:(j + 1) * CH],
                                             start=(k == 0), stop=(k == NK - 1))
                        zap = x[m][:, j * CH:(j + 1) * CH]
                        nc.vector.scalar_tensor_tensor(
                            zap, pso[:], ob[m][:, 0:1], zap,
                            OP.add, OP.add, accum_out=sums[m][:, j:j + 1])
                        junk = ozp.tile([128, CH], F32, tag="junk")
                        nc.scalar.activation(junk[:], zap, AF.Square,
                                             accum_out=sumsq[m][:, j:j + 1])
                # ---- group stats ----
                ssq = [ostp.tile([128, 2], F32, tag=f"ssq{k}", name=f"ssq{k}") for k in range(NK)]
                for k in range(NK):
                    nc.vector.tensor_reduce(ssq[k][:, 0:1], sums[k][:, 0:NCH],
                                            mybir.AxisListType.X, OP.add)
                    nc.vector.tensor_reduce(ssq[k][:, 1:2], sumsq[k][:, 0:NCH],
                                            mybir.AxisListType.X, OP.add)
                with tc.tile_pool(name="stps", bufs=1, space="PSUM") as stps:
                    psg = stps.tile([GN_GROUPS, 2], F32, tag="psg")
                    for k in range(NK):
                        nc.tensor.matmul(psg[:], gsel[k][:], ssq[k][:],
                                         start=(k == 0), stop=(k == NK - 1))
                    mv = ostp.tile([GN_GROUPS, 2], F32, tag="mv")
                    nc.vector.tensor_scalar(mv[:], psg[:], inv_n, None, OP.mult)
                    mu2 = ostp.tile([GN_GROUPS, 1], F32, tag="mu2")
                    nc.vector.tensor_tensor(mu2[:], mv[:, 0:1], mv[:, 0:1], OP.mult)
                    var = ostp.tile([GN_GROUPS, 1], F32, tag="var")
                    nc.vector.tensor_tensor(var[:], mv[:, 1:2], mu2[:], OP.subtract)
                    sd = ostp.tile([GN_GROUPS, 1], F32, tag="sd")
                    nc.scalar.activation(sd[:], var[:], AF.Sqrt, bias=epsb[:, 0:1])
                    rstd = ostp.tile([GN_GROUPS, 1], F32, tag="rstd")
                    nc.vector.reciprocal(rstd[:], sd[:])
                    mr = ostp.tile([GN_GROUPS, 2], F32, tag="mr")
                    nc.vector.tensor_copy(mr[:, 0:1], mv[:, 0:1])
                    nc.vector.tensor_copy(mr[:, 1:2], rstd[:])
                    # expand group stats to channels; fold gn affine
                    scale = [ostp.tile([128, 1], F32, tag=f"scale{k}", name=f"scale{k}") for k in range(NK)]
                    bias = [ostp.tile([128, 1], F32, tag=f"bias{k}", name=f"bias{k}") for k in range(NK)]
                    for k in range(NK):
                        pse = stps.tile([128, 2], F32, tag=f"pse{k}")
                        nc.tensor.matmul(pse[:], gexp[k][:], mr[:], start=True, stop=True)
                        muc = ostp.tile([128, 1], F32, tag=f"muc{k}")
                        rc = ostp.tile([128, 1], F32, tag=f"rc{k}")
                        nc.vector.tensor_copy(muc[:], pse[:, 0:1])
                        nc.vector.tensor_copy(rc[:], pse[:, 1:2])
                        nc.vector.tensor_tensor(scale[k][:], rc[:], gnw[k][:], OP.mult)
                        tmp = ostp.tile([128, 1], F32, tag=f"tmp{k}")
                        nc.vector.tensor_tensor(tmp[:], muc[:], scale[k][:], OP.mult)
                        nc.vector.tensor_tensor(bias[k][:], gnb[k][:], tmp[:], OP.subtract)

                    # ---- pass 2: affine on stored z (no matmuls) ----
                    N_AF_POOL = int(os.environ.get("K_AF_POOL", "22"))
                    N_AF_ACT = int(os.environ.get("K_AF_ACT", "20"))
                    for j in range(NCH):
                        for m in range(NK):
                            idx = (j * NK + m) % 64
                            zap = x[m][:, j * CH:(j + 1) * CH]
                            of = obp.tile([128, CH], DBF, tag="of",
                                          name="of")
                            if idx < N_AF_POOL:
                                nc.gpsimd.tensor_scalar(
                                    of[:], zap, scale[m][:, 0:1],
                                    bias[m][:, 0:1], OP.mult, OP.add)
                            elif idx < N_AF_POOL + N_AF_ACT:
                                nc.scalar.activation(of[:], zap, AF.Identity,
                                                     bias=bias[m][:, 0:1],
                                                     scale=scale[m][:, 0:1])
                            else:
                                nc.vector.tensor_scalar(
                                    of[:], zap, scale[m][:, 0:1],
                                    bias[m][:, 0:1], OP.mult, OP.add)
                            nc.sync.dma_start(
                                _dram_ap(out_d, m * 128 * HW + j * CH,
                                         [[HW, 128], [1, CH]]),
                                of[:])
    nc.compile()
    return nc


_CACHE = {}


def _get_program(beta, use_kbias):
    key = (float(beta), bool(use_kbias))
    if key not in _CACHE:
        _CACHE[key] = build_program(beta, use_kbias)
    return _CACHE[key]


def make_host_inputs(feature, boundary_logits, beta, W_in, b_in, W_s, b_s,
                     p_bias, out_w, out_b, gn_w, gn_b):
    wi_t = np.ascontiguousarray(
        np.transpose(np.asarray(W_in, np.float32), (0, 2, 1))).astype(BF)
    ws_t = np.ascontiguousarray(
        np.transpose(np.asarray(W_s, np.float32), (0, 2, 1))).astype(BF)
    wo_t = np.ascontiguousarray(np.asarray(out_w, np.float32).T).astype(BF)
    kbv = (np.asarray(b_in, np.float32) + np.asarray(b_s, np.float32)
           + np.asarray(p_bias, np.float32))
    use_kbias = bool(np.any(kbv != 0.0))
    cpg = C // GN_GROUPS
    gsel = np.zeros((2, 128, GN_GROUPS), np.float32)
    gexp = np.zeros((2, GN_GROUPS, 128), np.float32)
    for k in range(2):
        for p in range(128):
            g = (k * 128 + p) // cpg
            gsel[k, p, g] = 1.0
            gexp[k, g, p] = 1.0
    common = {
        "wi_t": wi_t, "ws_t": ws_t, "wo_t": wo_t,
        "ob": np.asarray(out_b, np.float32),
        "gnw": np.asarray(gn_w, np.float32),
        "gnb": np.asarray(gn_b, np.float32),
        "gsel": gsel, "gexp": gexp,
        "ident": np.eye(128, dtype=np.float32),
        "revid": np.eye(128, dtype=np.float32)[::-1].copy(),
    }
    if use_kbias:
        common["kb"] = kbv.astype(BF)
    B = np.asarray(feature).shape[0]
    in_maps = []
    for b in range(B):
        m = dict(common)
        m["xb"] = np.asarray(feature[b], np.float32).astype(BF)
        m["bl"] = np.asarray(boundary_logits[b], np.float32).reshape(S, S)
        in_maps.append(m)
    return in_maps, float(np.asarray(beta).reshape(-1)[0]), use_kbias


def kernel(feature, boundary_logits, beta, W_in, b_in, W_s, b_s, p_bias,
           out_w, out_b, gn_w, gn_b):
    feature = np.asarray(feature)
    B = feature.shape[0]
    in_maps, beta_v, use_kbias = make_host_inputs(
        feature, boundary_logits, beta, W_in, b_in, W_s, b_s, p_bias,
        out_w, out_b, gn_w, gn_b)
    nc = _get_program(beta_v, use_kbias)
    res = run_bass_kernel_spmd(nc, in_maps, core_ids=list(range(B)))
    out = np.stack([np.asarray(r["out"]) for r in res.results], axis=0)
    return out.astype(np.float32)



# revision 45
# speedup vs baseline: 1.0740x; 1.0466x over previous
"""Trainium2 Bass kernel for BoundaryFeaturePropagation.

Sharding: data-parallel over batch — one batch image per NeuronCore
(B=8 over 8 cores); the small [C,C] weights are replicated on all cores.

Per-core pipeline:
  1. gate:   conf = clip(1 - beta*sigmoid(a*sigmoid(bl) - g), 0, 1), plus
             PE transposes to build per-step gate rows for all 4 directions.
  2. scan:   4 directional gated RNN scans run concurrently, state layout
             [c(part), n(free)].  Per step: input-proj matmuls (batched in
             2-step PSUM blocks, bf16), state matmuls accumulate into the
             same PSUM bank, DVE scalar_tensor_tensor computes the next
             gated state hg = relu(psum)*g directly from PSUM (relu/gate
             commute since g>=0), ACT evacs h = relu(psum) into a history
             ring, and periodic block adds accumulate the ring into the
             fused accumulator.
  3. output: fused -> out_w projection (two passes: one for GroupNorm
             stats, one for normalized output), residual add from the
             bf16-resident input, GN affine folded into one ACT op.
All matmuls are bf16 with fp32 PSUM accumulation.
"""

import os
import sys

for _p in ("/opt/trn_rl_repo",):
    if _p not in sys.path and os.path.isdir(_p):
        sys.path.insert(0, _p)

import numpy as np
import ml_dtypes
from contextlib import ExitStack

import concourse.bass as bass
import concourse.bacc as bacc
import concourse.mybir as mybir
import concourse.tile as tile
from concourse import library_config
from concourse.bass_utils import run_bass_kernel_spmd

BF = ml_dtypes.bfloat16
F32 = mybir.dt.float32
DBF = mybir.dt.bfloat16
AF = mybir.ActivationFunctionType
OP = mybir.AluOpType

ALPHA = 20.0
GAMMA = 4.0
GN_GROUPS = 32
EPS = 1e-5

C = 256
NK = 2          # c-tiles (k/m halves of 128)
S = 128         # H = W
HW = S * S
DEPTH = 8       # history ring slots per direction
ABLK = 4        # steps per acc block-add
PBLK = 2        # steps per psum proj block
CH = 512        # output-phase chunk (positions)
NCH = HW // CH
HG_FROM_HIST = os.environ.get("K_HG_FROM_HIST", "0") == "1"


def _mkap(t, off, dims):
    """Custom free-dim AP on a tile: dims = [[step, count], ...] (outer->inner),
    off in elements of the tile's free space."""
    a = t[:]
    return bass.AP(a.tensor, a.offset + off, [list(a.ap[0])] + [list(d) for d in dims])


def _dram_ap(d, off, dims):
    a = d[:] if not isinstance(d, bass.AP) else d
    return bass.AP(a.tensor, off, [list(x) for x in dims])


def build_program(beta, use_kbias):
    """Build the SPMD single-core program (same on all 8 cores)."""
    nc = bacc.Bacc("TRN2", target_bir_lowering=False, debug=False)

    # ---- DRAM I/O ----
    xb_d = nc.dram_tensor("xb", [C, S, S], DBF, kind="ExternalInput")
    bl_d = nc.dram_tensor("bl", [S, S], F32, kind="ExternalInput")
    wi_d = nc.dram_tensor("wi_t", [4, C, C], DBF, kind="ExternalInput")
    ws_d = nc.dram_tensor("ws_t", [4, C, C], DBF, kind="ExternalInput")
    wo_d = nc.dram_tensor("wo_t", [C, C], DBF, kind="ExternalInput")
    ob_d = nc.dram_tensor("ob", [C], F32, kind="ExternalInput")
    gnw_d = nc.dram_tensor("gnw", [C], F32, kind="ExternalInput")
    gnb_d = nc.dram_tensor("gnb", [C], F32, kind="ExternalInput")
    gsel_d = nc.dram_tensor("gsel", [2, 128, GN_GROUPS], F32, kind="ExternalInput")
    gexp_d = nc.dram_tensor("gexp", [2, GN_GROUPS, 128], F32, kind="ExternalInput")
    id_d = nc.dram_tensor("ident", [128, 128], F32, kind="ExternalInput")
    rid_d = nc.dram_tensor("revid", [128, 128], F32, kind="ExternalInput")
    if use_kbias:
        kb_d = nc.dram_tensor("kb", [4, C], DBF, kind="ExternalInput")
    gd = nc.dram_tensor("gdram", [S, 4, S], DBF)
    out_d = nc.dram_tensor("out", [C, S, S], DBF, kind="ExternalOutput")

    with tile.TileContext(nc) as tc:
        with ExitStack() as ctx:
            cp = ctx.enter_context(tc.tile_pool(name="const", bufs=1))

            # ---- resident tensors ----
            x = [cp.tile([128, HW], DBF, tag=f"x{k}", name=f"x{k}") for k in range(NK)]
            acc = [cp.tile([128, HW], DBF, tag=f"acc{k}", name=f"acc{k}") for k in range(NK)]
            hist = [cp.tile([128, 4, DEPTH, S], DBF, tag=f"hist{k}", name=f"hist{k}") for k in range(NK)]
            wi = [cp.tile([128, 4, NK, 128], DBF, tag=f"wi{k}", name=f"wi{k}") for k in range(NK)]
            ws = [cp.tile([128, 4, NK, 128], DBF, tag=f"ws{k}", name=f"ws{k}") for k in range(NK)]
            wo = [cp.tile([128, NK, 128], DBF, tag=f"wo{k}", name=f"wo{k}") for k in range(NK)]
            G = cp.tile([128, 4, S], DBF, tag="G")            # gate rows per step t
            bl = cp.tile([128, S], F32, tag="bl")
            conf = cp.tile([128, S], F32, tag="conf")
            confT = cp.tile([128, S], F32, tag="confT")
            confTr = cp.tile([128, S], F32, tag="confTr")
            confr = cp.tile([128, S], F32, tag="confr")
            ident = cp.tile([128, 128], F32, tag="ident")
            revid = cp.tile([128, 128], F32, tag="revid")
            ob = [cp.tile([128, 1], F32, tag=f"ob{k}", name=f"ob{k}") for k in range(NK)]
            gnw = [cp.tile([128, 1], F32, tag=f"gnw{k}", name=f"gnw{k}") for k in range(NK)]
            gnb = [cp.tile([128, 1], F32, tag=f"gnb{k}", name=f"gnb{k}") for k in range(NK)]
            gsel = [cp.tile([128, GN_GROUPS], F32, tag=f"gsel{k}", name=f"gsel{k}") for k in range(NK)]
            gexp = [cp.tile([GN_GROUPS, 128], F32, tag=f"gexp{k}", name=f"gexp{k}") for k in range(NK)]
            sums = [cp.tile([128, NCH], F32, tag=f"sums{k}", name=f"sums{k}") for k in range(NK)]
            sumsq = [cp.tile([128, NCH], F32, tag=f"sumsq{k}", name=f"sumsq{k}") for k in range(NK)]
            if use_kbias:
                kb = cp.tile([1, 4 * C], DBF, tag="kb")
                ones_row = cp.tile([1, PBLK * S], DBF, tag="ones_row")
                nc.vector.memset(ones_row[:], 1.0)

            # ---- DMAs in ----
            NXC = 4
            for k in range(NK):
                for j in range(NXC):
                    sz = HW // NXC
                    nc.sync.dma_start(
                        x[k][:, j * sz:(j + 1) * sz],
                        _dram_ap(xb_d, k * 128 * HW + j * sz,
                                 [[HW, 128], [1, sz]]))
                nc.sync.dma_start(
                    wi[k][:],
                    _dram_ap(wi_d, k * 128 * C,
                             [[C, 128], [C * C, 4], [128, NK], [1, 128]]))
                nc.sync.dma_start(
                    ws[k][:],
                    _dram_ap(ws_d, k * 128 * C,
                             [[C, 128], [C * C, 4], [128, NK], [1, 128]]))
                nc.sync.dma_start(
                    wo[k][:], _dram_ap(wo_d, k * 128 * C,
                                       [[C, 128], [128, NK], [1, 128]]))
                nc.sync.dma_start(ob[k][:], _dram_ap(ob_d, k * 128, [[1, 128], [1, 1]]))
                nc.sync.dma_start(gnw[k][:], _dram_ap(gnw_d, k * 128, [[1, 128], [1, 1]]))
                nc.sync.dma_start(gnb[k][:], _dram_ap(gnb_d, k * 128, [[1, 128], [1, 1]]))
                nc.sync.dma_start(gsel[k][:], _dram_ap(gsel_d, k * 128 * GN_GROUPS,
                                                       [[GN_GROUPS, 128], [1, GN_GROUPS]]))
                nc.sync.dma_start(gexp[k][:], _dram_ap(gexp_d, k * GN_GROUPS * 128,
                                                       [[128, GN_GROUPS], [1, 128]]))
            with tc.high_priority():
                nc.sync.dma_start(bl[:, :], _dram_ap(bl_d, 0, [[S, 128], [1, S]]))
                nc.sync.dma_start(ident[:], id_d[:])
                nc.sync.dma_start(revid[:], rid_d[:])
            if use_kbias:
                nc.sync.dma_start(kb[:], _dram_ap(kb_d, 0, [[4 * C, 1], [1, 4 * C]]))

            # ---- gate computation (prioritized: it gates the scan) ----
            ctx_g = tc.high_priority()
            ctx_g.__enter__()
            s1 = cp.tile([128, S], F32, tag="s1")
            ngam = cp.tile([128, 1], F32, tag="ngam")
            nc.vector.memset(ngam[:], -GAMMA)
            epsb = cp.tile([GN_GROUPS, 1], F32, tag="epsb")
            nc.vector.memset(epsb[:], EPS)
            nc.scalar.activation(s1[:], bl[:], AF.Sigmoid)
            nc.scalar.activation(conf[:], s1[:], AF.Sigmoid, bias=ngam[:, 0:1],
                                 scale=ALPHA)
            # conf = clip(1 - beta*s2, 0, 1)
            nc.vector.tensor_scalar(conf[:], conf[:], -float(beta), 1.0, OP.mult, OP.add)
            nc.vector.tensor_scalar(conf[:], conf[:], 0.0, 1.0, OP.max, OP.min)

            # transposes: confT[t,n]=conf[n,t]; partition reversals via the
            # anti-diagonal permutation: rev(M) = revid.T @ M.
            # confr[t,n]=conf[S-1-t,n]; confTr[t,n]=conf[n,S-1-t]=confT[S-1-t,n]
            with tc.tile_pool(name="tp_ps", bufs=1, space="PSUM") as tps:
                pt = tps.tile([128, 128], F32, tag="pt")
                nc.tensor.transpose(pt[:], conf[:], ident[:])
                nc.vector.tensor_copy(confT[:], pt[:])
                pt2 = tps.tile([128, 128], F32, tag="pt2")
                nc.tensor.matmul(pt2[:], revid[:], conf[:], start=True, stop=True)
                nc.vector.tensor_copy(confr[:], pt2[:])
                pt3 = tps.tile([128, 128], F32, tag="pt3")
                nc.tensor.matmul(pt3[:], revid[:], confT[:], start=True, stop=True)
                nc.vector.tensor_copy(confTr[:], pt3[:])

            # assemble G[t, dir, n] (bf16): lr=confT, rl=confTr, tb=conf, bt=confr
            nc.vector.tensor_copy(G[:, 0, :], confT[:])
            nc.vector.tensor_copy(G[:, 1, :], confTr[:])
            nc.vector.tensor_copy(G[:, 2, :], conf[:])
            nc.vector.tensor_copy(G[:, 3, :], confr[:])
            # stage gate table to DRAM for per-step stride-0 broadcast reads
            nc.sync.dma_start(_dram_ap(gd, 0, [[4 * S, 128], [1, 4 * S]]), G[:])
            ctx_g.__exit__(None, None, None)

            ctx_ms = tc.high_priority(offset=-192)
            ctx_ms.__enter__()
            for k in range(NK):
                nc.vector.memset(acc[k][:, 0:HW // 2], 0.0)
                nc.gpsimd.memset(acc[k][:, HW // 2:], 0.0)
            ctx_ms.__exit__(None, None, None)

            # ================= SCAN =================
            # direction d: 0=lr, 1=rl, 2=tb, 3=bt
            # lr/tb get one PBLK-step proj matmul; rl/bt (reversed walks)
            # get per-step matmuls so all APs keep positive strides.
            def proj_rhs_blk(k, d, t0):
                if d == 0:
                    return _mkap(x[k], t0, [[1, PBLK], [S, S]])
                return _mkap(x[k], t0 * S, [[S, PBLK], [1, S]])

            def proj_rhs_step(k, d, t):
                if d == 1:
                    return _mkap(x[k], S - 1 - t, [[S, S]])
                return _mkap(x[k], (S - 1 - t) * S, [[1, S]])

            # zero-region (2KB) boundaries inside the [128, 4, PBLK*S] psum tile:
            dir_bytes = PBLK * S * 4
            first_in_zr = [d for d in range(4) if (d * dir_bytes) % 2048 == 0]
            last_in_zr = [d for d in range(4)
                          if ((d + 1) * dir_bytes) % 2048 == 0 or d == 3]

            gbp = ctx.enter_context(tc.tile_pool(name="gb", bufs=int(os.environ.get("K_GB", "10"))))
            hgp = ctx.enter_context(tc.tile_pool(name="hg", bufs=int(os.environ.get("K_HG", "4"))))
            with ExitStack() as sctx:
                psp = [sctx.enter_context(
                        tc.tile_pool(name=f"ps{m}", bufs=2, space="PSUM"))
                       for m in range(NK)]
                nblocks = S // PBLK
                hg_cur = None
                ps_tiles = {}

                def alloc_ps(b):
                    if b not in ps_tiles and b < nblocks:
                        ps_tiles[b] = [psp[m].tile([128, 4, PBLK * S], F32,
                                                   tag=f"psb{m}", name=f"psb{m}")
                                       for m in range(NK)]
                    return ps_tiles.get(b)

                def emit_proj(b, m):
                    """Input-projection matmuls for block b, output half m."""
                    if b >= nblocks:
                        return
                    t0 = PBLK * b
                    ps = ps_tiles[b]
                    for d in range(4):
                        for k in range(NK):
                            if d in (0, 2):
                                nc.tensor.matmul(
                                    ps[m][:, d, :],
                                    wi[k][:, d, m, :],
                                    proj_rhs_blk(k, d, t0),
                                    start=(k == 0 and d in (0, 2)),
                                    stop=False, skip_group_check=True)
                            else:
                                for ti in range(PBLK):
                                    nc.tensor.matmul(
                                        ps[m][:, d, ti * S:(ti + 1) * S],
                                        wi[k][:, d, m, :],
                                        proj_rhs_step(k, d, t0 + ti),
                                        start=False,
                                        stop=False, skip_group_check=True)
                        if use_kbias:
                            nc.tensor.matmul(
                                ps[m][:, d, :],
                                kb[:, d * C + m * 128: d * C + (m + 1) * 128],
                                ones_row[:],
                                start=False, stop=False, skip_group_check=True)

                # prefetch ALL per-step gate rows up front; the pool's
                # slot reuse (bufs) self-paces the DMAs against consumers
                gb_tiles = {}
                for tt in range(1, S):
                    g_t = gbp.tile([128, 4, S], DBF, tag="gb", name="gb")
                    nc.sync.dma_start(
                        g_t[:], _dram_ap(gd, tt * 4 * S,
                                         [[0, 128], [1, 4 * S]]))
                    gb_tiles[tt] = g_t

                alloc_ps(0)
                for m in range(NK):
                    emit_proj(0, m)
                for b in range(nblocks):
                    t0 = PBLK * b
                    ps = ps_tiles.pop(b)
                    alloc_ps(b + 1)
                    for ti in range(PBLK):
                        t = t0 + ti
                        slot = t % DEPTH
                        # --- state matmuls (skip t=0), m-major: ps[0] is
                        # complete after the first half, so the m0 gate op
                        # overlaps the m1 matmuls; next step's k-consumers
                        # of hg[0] come first so hg[1] can still be in flight ---
                        if t + 1 < S:
                            gb = gb_tiles.pop(t + 1)
                            hg_nxt = [hgp.tile([128, 4, S], DBF, tag=f"hg{m}",
                                               name=f"hg{m}")
                                      for m in range(NK)]
                        else:
                            gb = None
                        for m in range(NK):
                            if t > 0:
                                for k in range(NK):
                                    for d in range(4):
                                        nc.tensor.matmul(
                                            ps[m][:, d, ti * S:(ti + 1) * S],
                                            ws[k][:, d, m, :],
                                            hg_cur[k][:, d, :],
                                            start=False,
                                            stop=(ti == PBLK - 1 and k == NK - 1
                                                  and d % 2 == 1),
                                            skip_group_check=True)
                            # gate op for this half fires as soon as its own
                            # matmuls are done (boosted: heads the DVE queue)
                            if gb is not None:
                                with tc.high_priority(offset=int(os.environ.get("K_STTP", "0"))):
                                    nc.vector.scalar_tensor_tensor(
                                        hg_nxt[m][:], ps[m][:, :, ti * S:(ti + 1) * S],
                                        0.0, gb[:], OP.max, OP.mult)
                        if gb is not None:
                            hg_cur = hg_nxt
                        # next block's proj fills the PE gap during the gate op
                        emit_proj(b + 1, ti)
                        # --- history evac: h = relu(psum), both block steps in
                        # one ACT op per m-half ---
                        if ti == PBLK - 1:
                            for m in range(NK):
                                nc.scalar.activation(
                                    hist[m][:, :, slot - 1:slot + 1, :],
                                    ps[m][:, :, :], AF.Relu)
                        # --- acc block adds every ABLK steps (gpsimd;
                        # deprioritized so the gate op wins the queues) ---
                        if t % ABLK == ABLK - 1:
                            tb0 = t - (ABLK - 1)
                            s0 = tb0 % DEPTH
                            ctx_p = tc.high_priority(offset=-int(os.environ.get('K_PRIO', '128')))
                            ctx_p.__enter__()
                            for k in range(NK):
                                # lr: cols tb0..t (h outer, w inner)
                                nc.gpsimd.tensor_tensor(
                                    _mkap(acc[k], tb0, [[S, S], [1, ABLK]]),
                                    _mkap(acc[k], tb0, [[S, S], [1, ABLK]]),
                                    _mkap(hist[k], (0 * DEPTH + s0) * S,
                                          [[1, S], [S, ABLK]]),
                                    OP.add)
                                # rl: cols S-1-tb0 down
                                nc.vector.tensor_tensor(
                                    _mkap(acc[k], S - 1 - tb0, [[S, S], [-1, ABLK]]),
                                    _mkap(acc[k], S - 1 - tb0, [[S, S], [-1, ABLK]]),
                                    _mkap(hist[k], (1 * DEPTH + s0) * S,
                                          [[1, S], [S, ABLK]]),
                                    OP.add)
                                # tb: rows tb0..t (slot outer, w inner)
                                nc.gpsimd.tensor_tensor(
                                    _mkap(acc[k], tb0 * S, [[S, ABLK], [1, S]]),
                                    _mkap(acc[k], tb0 * S, [[S, ABLK], [1, S]]),
                                    hist[k][:, 2, s0:s0 + ABLK, :],
                                    OP.add)
                                # bt: rows S-1-tb0 down
                                nc.vector.tensor_tensor(
                                    _mkap(acc[k], (S - 1 - tb0) * S, [[-S, ABLK], [1, S]]),
                                    _mkap(acc[k], (S - 1 - tb0) * S, [[-S, ABLK], [1, S]]),
                                    hist[k][:, 3, s0:s0 + ABLK, :],
                                    OP.add)
                            ctx_p.__exit__(None, None, None)

            # ================= OUTPUT =================
            inv_n = 1.0 / (8.0 * HW)
            with (
                tc.tile_pool(name="ops", bufs=3, space="PSUM") as ops_pool,
                tc.tile_pool(name="oz", bufs=2) as ozp,
                tc.tile_pool(name="ost", bufs=1) as ostp,
                tc.tile_pool(name="obuf", bufs=3) as obp,
            ):
                # ---- pass 1: matmuls + z (bf16, in-place over x) + stats ----
                for j in range(NCH):
                    for m in range(NK):
                        pso = ops_pool.tile([128, CH], F32, tag=f"pso{m}")
                        for k in range(NK):
                            nc.tensor.matmul(pso[:], wo[k][:, m, :],
                                             acc[k][:, j * CH:(j + 1) * CH],
                                             start=(k == 0), stop=(k == NK - 1))
                        zap = x[m][:, j * CH:(j + 1) * CH]
                        nc.vector.scalar_tensor_tensor(
                            zap, pso[:], ob[m][:, 0:1], zap,
                            OP.add, OP.add, accum_out=sums[m][:, j:j + 1])
                        junk = ozp.tile([128, CH], F32, tag="junk")
                        nc.scalar.activation(junk[:], zap, AF.Square,
                                             accum_out=sumsq[m][:, j:j + 1])
                # ---- group stats ----
                ssq = [ostp.tile([128, 2], F32, tag=f"ssq{k}", name=f"ssq{k}") for k in range(NK)]
                for k in range(NK):
                    nc.vector.tensor_reduce(ssq[k][:, 0:1], sums[k][:, 0:NCH],
                                            mybir.AxisListType.X, OP.add)
                    nc.vector.tensor_reduce(ssq[k][:, 1:2], sumsq[k][:, 0:NCH],
                                            mybir.AxisListType.X, OP.add)
                with tc.tile_pool(name="stps", bufs=1, space="PSUM") as stps:
                    psg = stps.tile([GN_GROUPS, 2], F32, tag="psg")
                    for k in range(NK):
                        nc.tensor.matmul(psg[:], gsel[k][:], ssq[k][:],
                                         start=(k == 0), stop=(k == NK - 1))
                    mv = ostp.tile([GN_GROUPS, 2], F32, tag="mv")
                    nc.vector.tensor_scalar(mv[:], psg[:], inv_n, None, OP.mult)
                    mu2 = ostp.tile([GN_GROUPS, 1], F32, tag="mu2")
                    nc.vector.tensor_tensor(mu2[:], mv[:, 0:1], mv[:, 0:1], OP.mult)
                    var = ostp.tile([GN_GROUPS, 1], F32, tag="var")
                    nc.vector.tensor_tensor(var[:], mv[:, 1:2], mu2[:], OP.subtract)
                    sd = ostp.tile([GN_GROUPS, 1], F32, tag="sd")
                    nc.scalar.activation(sd[:], var[:], AF.Sqrt, bias=epsb[:, 0:1])
                    rstd = ostp.tile([GN_GROUPS, 1], F32, tag="rstd")
                    nc.vector.reciprocal(rstd[:], sd[:])
                    mr = ostp.tile([GN_GROUPS, 2], F32, tag="mr")
                    nc.vector.tensor_copy(mr[:, 0:1], mv[:, 0:1])
                    nc.vector.tensor_copy(mr[:, 1:2], rstd[:])
                    # expand group stats to channels; fold gn affine
                    scale = [ostp.tile([128, 1], F32, tag=f"scale{k}", name=f"scale{k}") for k in range(NK)]
                    bias = [ostp.tile([128, 1], F32, tag=f"bias{k}", name=f"bias{k}") for k in range(NK)]
                    pse = stps.tile([128, 2, 2], F32, tag="pse")
                    for k in range(NK):
                        nc.tensor.matmul(pse[:, k, :], gexp[k][:], mr[:],
                                         start=(k == 0), stop=(k == NK - 1),
                                         skip_group_check=True)
                    for k in range(NK):
                        muc = ostp.tile([128, 1], F32, tag=f"muc{k}")
                        rc = ostp.tile([128, 1], F32, tag=f"rc{k}")
                        nc.vector.tensor_copy(muc[:], pse[:, k, 0:1])
                        nc.vector.tensor_copy(rc[:], pse[:, k, 1:2])
                        nc.vector.tensor_tensor(scale[k][:], rc[:], gnw[k][:], OP.mult)
                        tmp = ostp.tile([128, 1], F32, tag=f"tmp{k}")
                        nc.vector.tensor_tensor(tmp[:], muc[:], scale[k][:], OP.mult)
                        nc.vector.tensor_tensor(bias[k][:], gnb[k][:], tmp[:], OP.subtract)

                    # ---- pass 2: affine on stored z; 4-chunk staging
                    # tiles so each out DMA covers 2048 positions (fewer
                    # HWDGE descriptor-gen slots) ----
                    N_AF_POOL = int(os.environ.get("K_AF_POOL", "22"))
                    N_AF_ACT = int(os.environ.get("K_AF_ACT", "20"))
                    OB = 4
                    u = 0
                    for jb in range(0, NCH, OB):
                        for m in range(NK):
                            of = obp.tile([128, OB * CH], DBF, tag="of",
                                          name="of")
                            for q in range(OB):
                                j = jb + q
                                idx = u % 64
                                u += 1
                                zap = x[m][:, j * CH:(j + 1) * CH]
                                oq = of[:, q * CH:(q + 1) * CH]
                                if idx < N_AF_POOL:
                                    nc.gpsimd.tensor_scalar(
                                        oq, zap, scale[m][:, 0:1],
                                        bias[m][:, 0:1], OP.mult, OP.add)
                                elif idx < N_AF_POOL + N_AF_ACT:
                                    nc.scalar.activation(
                                        oq, zap, AF.Identity,
                                        bias=bias[m][:, 0:1],
                                        scale=scale[m][:, 0:1])
                                else:
                                    nc.vector.tensor_scalar(
                                        oq, zap, scale[m][:, 0:1],
                                        bias[m][:, 0:1], OP.mult, OP.add)
                            nc.sync.dma_start(
                                _dram_ap(out_d, m * 128 * HW + jb * CH,
                                         [[HW, 128], [1, OB * CH]]),
                                of[:])
    nc.compile()
    return nc


_CACHE = {}


def _get_program(beta, use_kbias):
    key = (float(beta), bool(use_kbias))
    if key not in _CACHE:
        _CACHE[key] = build_program(beta, use_kbias)
    return _CACHE[key]


def make_host_inputs(feature, boundary_logits, beta, W_in, b_in, W_s, b_s,
                     p_bias, out_w, out_b, gn_w, gn_b):
    wi_t = np.ascontiguousarray(
        np.transpose(np.asarray(W_in, np.float32), (0, 2, 1))).astype(BF)
    ws_t = np.ascontiguousarray(
        np.transpose(np.asarray(W_s, np.float32), (0, 2, 1))).astype(BF)
    wo_t = np.ascontiguousarray(np.asarray(out_w, np.float32).T).astype(BF)
    kbv = (np.asarray(b_in, np.float32) + np.asarray(b_s, np.float32)
           + np.asarray(p_bias, np.float32))
    use_kbias = bool(np.any(kbv != 0.0))
    cpg = C // GN_GROUPS
    gsel = np.zeros((2, 128, GN_GROUPS), np.float32)
    gexp = np.zeros((2, GN_GROUPS, 128), np.float32)
    for k in range(2):
        for p in range(128):
            g = (k * 128 + p) // cpg
            gsel[k, p, g] = 1.0
            gexp[k, g, p] = 1.0
    common = {
        "wi_t": wi_t, "ws_t": ws_t, "wo_t": wo_t,
        "ob": np.asarray(out_b, np.float32),
        "gnw": np.asarray(gn_w, np.float32),
        "gnb": np.asarray(gn_b, np.float32),
        "gsel": gsel, "gexp": gexp,
        "ident": np.eye(128, dtype=np.float32),
        "revid": np.eye(128, dtype=np.float32)[::-1].copy(),
    }
    if use_kbias:
        common["kb"] = kbv.astype(BF)
    B = np.asarray(feature).shape[0]
    in_maps = []
    for b in range(B):
        m = dict(common)
        m["xb"] = np.asarray(feature[b], np.float32).astype(BF)
        m["bl"] = np.asarray(boundary_logits[b], np.float32).reshape(S, S)
        in_maps.append(m)
    return in_maps, float(np.asarray(beta).reshape(-1)[0]), use_kbias


def kernel(feature, boundary_logits, beta, W_in, b_in, W_s, b_s, p_bias,
           out_w, out_b, gn_w, gn_b):
    feature = np.asarray(feature)
    B = feature.shape[0]
    in_maps, beta_v, use_kbias = make_host_inputs(
        feature, boundary_logits, beta, W_in, b_in, W_s, b_s, p_bias,
        out_w, out_b, gn_w, gn_b)
    nc = _get_program(beta_v, use_kbias)
    res = run_bass_kernel_spmd(nc, in_maps, core_ids=list(range(B)))
    out = np.stack([np.asarray(r["out"]) for r in res.results], axis=0)
    return out.astype(np.float32)



# revision 47
# speedup vs baseline: 1.1131x; 1.0364x over previous
"""Trainium2 Bass kernel for BoundaryFeaturePropagation.

Sharding: data-parallel over batch — one batch image per NeuronCore
(B=8 over 8 cores); the small [C,C] weights are replicated on all cores.

Per-core pipeline:
  1. gate:   conf = clip(1 - beta*sigmoid(a*sigmoid(bl) - g), 0, 1), plus
             PE transposes to build per-step gate rows for all 4 directions.
  2. scan:   4 directional gated RNN scans run concurrently, state layout
             [c(part), n(free)].  Per step: input-proj matmuls (batched in
             2-step PSUM blocks, bf16), state matmuls accumulate into the
             same PSUM bank, DVE scalar_tensor_tensor computes the next
             gated state hg = relu(psum)*g directly from PSUM (relu/gate
             commute since g>=0), ACT evacs h = relu(psum) into a history
             ring, and periodic block adds accumulate the ring into the
             fused accumulator.
  3. output: fused -> out_w projection (two passes: one for GroupNorm
             stats, one for normalized output), residual add from the
             bf16-resident input, GN affine folded into one ACT op.
All matmuls are bf16 with fp32 PSUM accumulation.
"""

import os
import sys

for _p in ("/opt/trn_rl_repo",):
    if _p not in sys.path and os.path.isdir(_p):
        sys.path.insert(0, _p)

import numpy as np
import ml_dtypes
from contextlib import ExitStack

import concourse.bass as bass
import concourse.bacc as bacc
import concourse.mybir as mybir
import concourse.tile as tile
from concourse import library_config
from concourse.bass_utils import run_bass_kernel_spmd

BF = ml_dtypes.bfloat16
F32 = mybir.dt.float32
DBF = mybir.dt.bfloat16
AF = mybir.ActivationFunctionType
OP = mybir.AluOpType

ALPHA = 20.0
GAMMA = 4.0
GN_GROUPS = 32
EPS = 1e-5

C = 256
NK = 2          # c-tiles (k/m halves of 128)
S = 128         # H = W
HW = S * S
DEPTH = 8       # history ring slots per direction
ABLK = 4        # steps per acc block-add
PBLK = 2        # steps per psum proj block
CH = 512        # output-phase chunk (positions)
NCH = HW // CH
HG_FROM_HIST = os.environ.get("K_HG_FROM_HIST", "0") == "1"


def _mkap(t, off, dims):
    """Custom free-dim AP on a tile: dims = [[step, count], ...] (outer->inner),
    off in elements of the tile's free space."""
    a = t[:]
    return bass.AP(a.tensor, a.offset + off, [list(a.ap[0])] + [list(d) for d in dims])


def _dram_ap(d, off, dims):
    a = d[:] if not isinstance(d, bass.AP) else d
    return bass.AP(a.tensor, off, [list(x) for x in dims])


def build_program(beta, use_kbias):
    """Build the SPMD single-core program (same on all 8 cores)."""
    nc = bacc.Bacc("TRN2", target_bir_lowering=False, debug=False)

    # ---- DRAM I/O ----
    xb_d = nc.dram_tensor("xb", [C, S, S], DBF, kind="ExternalInput")
    bl_d = nc.dram_tensor("bl", [S, S], F32, kind="ExternalInput")
    wi_d = nc.dram_tensor("wi_t", [4, C, C], DBF, kind="ExternalInput")
    ws_d = nc.dram_tensor("ws_t", [4, C, C], DBF, kind="ExternalInput")
    wo_d = nc.dram_tensor("wo_t", [C, C], DBF, kind="ExternalInput")
    ob_d = nc.dram_tensor("ob", [C], F32, kind="ExternalInput")
    gnw_d = nc.dram_tensor("gnw", [C], F32, kind="ExternalInput")
    gnb_d = nc.dram_tensor("gnb", [C], F32, kind="ExternalInput")
    gsel_d = nc.dram_tensor("gsel", [2, 128, GN_GROUPS], F32, kind="ExternalInput")
    gexp_d = nc.dram_tensor("gexp", [2, GN_GROUPS, 128], F32, kind="ExternalInput")
    id_d = nc.dram_tensor("ident", [128, 128], F32, kind="ExternalInput")
    rid_d = nc.dram_tensor("revid", [128, 128], F32, kind="ExternalInput")
    if use_kbias:
        kb_d = nc.dram_tensor("kb", [4, C], DBF, kind="ExternalInput")
    gd = nc.dram_tensor("gdram", [S, 4, S], DBF)
    out_d = nc.dram_tensor("out", [C, S, S], DBF, kind="ExternalOutput")

    with tile.TileContext(nc) as tc:
        with ExitStack() as ctx:
            cp = ctx.enter_context(tc.tile_pool(name="const", bufs=1))

            # ---- resident tensors ----
            x = [cp.tile([128, HW], DBF, tag=f"x{k}", name=f"x{k}") for k in range(NK)]
            acc = [cp.tile([128, HW], DBF, tag=f"acc{k}", name=f"acc{k}") for k in range(NK)]
            hist = [cp.tile([128, 4, DEPTH, S], DBF, tag=f"hist{k}", name=f"hist{k}") for k in range(NK)]
            wi = [cp.tile([128, 4, NK, 128], DBF, tag=f"wi{k}", name=f"wi{k}") for k in range(NK)]
            ws = [cp.tile([128, 4, NK, 128], DBF, tag=f"ws{k}", name=f"ws{k}") for k in range(NK)]
            wo = [cp.tile([128, NK, 128], DBF, tag=f"wo{k}", name=f"wo{k}") for k in range(NK)]
            G = cp.tile([128, 4, S], DBF, tag="G")            # gate rows per step t
            bl = cp.tile([128, S], F32, tag="bl")
            conf = cp.tile([128, S], F32, tag="conf")
            confT = cp.tile([128, S], F32, tag="confT")
            confTr = cp.tile([128, S], F32, tag="confTr")
            confr = cp.tile([128, S], F32, tag="confr")
            ident = cp.tile([128, 128], F32, tag="ident")
            revid = cp.tile([128, 128], F32, tag="revid")
            ob = [cp.tile([128, 1], F32, tag=f"ob{k}", name=f"ob{k}") for k in range(NK)]
            gnw = [cp.tile([128, 1], F32, tag=f"gnw{k}", name=f"gnw{k}") for k in range(NK)]
            gnb = [cp.tile([128, 1], F32, tag=f"gnb{k}", name=f"gnb{k}") for k in range(NK)]
            gsel = [cp.tile([128, GN_GROUPS], F32, tag=f"gsel{k}", name=f"gsel{k}") for k in range(NK)]
            gexp = [cp.tile([GN_GROUPS, 128], F32, tag=f"gexp{k}", name=f"gexp{k}") for k in range(NK)]
            sums = [cp.tile([128, NCH], F32, tag=f"sums{k}", name=f"sums{k}") for k in range(NK)]
            sumsq = [cp.tile([128, NCH], F32, tag=f"sumsq{k}", name=f"sumsq{k}") for k in range(NK)]
            if use_kbias:
                kb = cp.tile([1, 4 * C], DBF, tag="kb")
                ones_row = cp.tile([1, PBLK * S], DBF, tag="ones_row")
                nc.vector.memset(ones_row[:], 1.0)

            # ---- DMAs in ----
            NXC = 4
            for k in range(NK):
                for j in range(NXC):
                    sz = HW // NXC
                    nc.sync.dma_start(
                        x[k][:, j * sz:(j + 1) * sz],
                        _dram_ap(xb_d, k * 128 * HW + j * sz,
                                 [[HW, 128], [1, sz]]))
                nc.sync.dma_start(
                    wi[k][:],
                    _dram_ap(wi_d, k * 128 * C,
                             [[C, 128], [C * C, 4], [128, NK], [1, 128]]))
                nc.sync.dma_start(
                    ws[k][:],
                    _dram_ap(ws_d, k * 128 * C,
                             [[C, 128], [C * C, 4], [128, NK], [1, 128]]))
                nc.sync.dma_start(
                    wo[k][:], _dram_ap(wo_d, k * 128 * C,
                                       [[C, 128], [128, NK], [1, 128]]))
                nc.sync.dma_start(ob[k][:], _dram_ap(ob_d, k * 128, [[1, 128], [1, 1]]))
                nc.sync.dma_start(gnw[k][:], _dram_ap(gnw_d, k * 128, [[1, 128], [1, 1]]))
                nc.sync.dma_start(gnb[k][:], _dram_ap(gnb_d, k * 128, [[1, 128], [1, 1]]))
                nc.sync.dma_start(gsel[k][:], _dram_ap(gsel_d, k * 128 * GN_GROUPS,
                                                       [[GN_GROUPS, 128], [1, GN_GROUPS]]))
                nc.sync.dma_start(gexp[k][:], _dram_ap(gexp_d, k * GN_GROUPS * 128,
                                                       [[128, GN_GROUPS], [1, 128]]))
            with tc.high_priority():
                nc.sync.dma_start(bl[:, :], _dram_ap(bl_d, 0, [[S, 128], [1, S]]))
                nc.sync.dma_start(ident[:], id_d[:])
                nc.sync.dma_start(revid[:], rid_d[:])
            if use_kbias:
                nc.sync.dma_start(kb[:], _dram_ap(kb_d, 0, [[4 * C, 1], [1, 4 * C]]))

            # ---- gate computation (prioritized: it gates the scan) ----
            ctx_g = tc.high_priority()
            ctx_g.__enter__()
            s1 = cp.tile([128, S], F32, tag="s1")
            ngam = cp.tile([128, 1], F32, tag="ngam")
            nc.vector.memset(ngam[:], -GAMMA)
            epsb = cp.tile([GN_GROUPS, 1], F32, tag="epsb")
            nc.vector.memset(epsb[:], EPS)
            nc.scalar.activation(s1[:], bl[:], AF.Sigmoid)
            nc.scalar.activation(conf[:], s1[:], AF.Sigmoid, bias=ngam[:, 0:1],
                                 scale=ALPHA)
            # conf = clip(1 - beta*s2, 0, 1)
            nc.vector.tensor_scalar(conf[:], conf[:], -float(beta), 1.0, OP.mult, OP.add)
            nc.vector.tensor_scalar(conf[:], conf[:], 0.0, 1.0, OP.max, OP.min)

            # transposes: confT[t,n]=conf[n,t]; partition reversals via the
            # anti-diagonal permutation: rev(M) = revid.T @ M.
            # confr[t,n]=conf[S-1-t,n]; confTr[t,n]=conf[n,S-1-t]=confT[S-1-t,n]
            with tc.tile_pool(name="tp_ps", bufs=1, space="PSUM") as tps:
                pt = tps.tile([128, 128], F32, tag="pt")
                nc.tensor.transpose(pt[:], conf[:], ident[:])
                nc.vector.tensor_copy(confT[:], pt[:])
                pt2 = tps.tile([128, 128], F32, tag="pt2")
                nc.tensor.matmul(pt2[:], revid[:], conf[:], start=True, stop=True)
                nc.vector.tensor_copy(confr[:], pt2[:])
                pt3 = tps.tile([128, 128], F32, tag="pt3")
                nc.tensor.matmul(pt3[:], revid[:], confT[:], start=True, stop=True)
                nc.vector.tensor_copy(confTr[:], pt3[:])

            # assemble G[t, dir, n] (bf16): lr=confT, rl=confTr, tb=conf, bt=confr
            nc.vector.tensor_copy(G[:, 0, :], confT[:])
            nc.vector.tensor_copy(G[:, 1, :], confTr[:])
            nc.vector.tensor_copy(G[:, 2, :], conf[:])
            nc.vector.tensor_copy(G[:, 3, :], confr[:])
            # stage gate table to DRAM for per-step stride-0 broadcast reads
            nc.sync.dma_start(_dram_ap(gd, 0, [[4 * S, 128], [1, 4 * S]]), G[:])
            ctx_g.__exit__(None, None, None)

            ctx_ms = tc.high_priority(offset=-192)
            ctx_ms.__enter__()
            for k in range(NK):
                nc.vector.memset(acc[k][:, 0:HW // 2], 0.0)
                nc.gpsimd.memset(acc[k][:, HW // 2:], 0.0)
            ctx_ms.__exit__(None, None, None)

            # ================= SCAN =================
            # direction d: 0=lr, 1=rl, 2=tb, 3=bt
            # lr/tb get one PBLK-step proj matmul; rl/bt (reversed walks)
            # get per-step matmuls so all APs keep positive strides.
            def proj_rhs_blk(k, d, t0):
                if d == 0:
                    return _mkap(x[k], t0, [[1, PBLK], [S, S]])
                return _mkap(x[k], t0 * S, [[S, PBLK], [1, S]])

            def proj_rhs_step(k, d, t):
                if d == 1:
                    return _mkap(x[k], S - 1 - t, [[S, S]])
                return _mkap(x[k], (S - 1 - t) * S, [[1, S]])

            # zero-region (2KB) boundaries inside the [128, 4, PBLK*S] psum tile:
            dir_bytes = PBLK * S * 4
            first_in_zr = [d for d in range(4) if (d * dir_bytes) % 2048 == 0]
            last_in_zr = [d for d in range(4)
                          if ((d + 1) * dir_bytes) % 2048 == 0 or d == 3]

            gbp = ctx.enter_context(tc.tile_pool(name="gb", bufs=int(os.environ.get("K_GB", "10"))))
            hgp = ctx.enter_context(tc.tile_pool(name="hg", bufs=int(os.environ.get("K_HG", "4"))))
            with ExitStack() as sctx:
                psp = [sctx.enter_context(
                        tc.tile_pool(name=f"ps{m}", bufs=2, space="PSUM"))
                       for m in range(NK)]
                nblocks = S // PBLK
                hg_cur = None
                ps_tiles = {}

                def alloc_ps(b):
                    if b not in ps_tiles and b < nblocks:
                        ps_tiles[b] = [psp[m].tile([128, 4, PBLK * S], F32,
                                                   tag=f"psb{m}", name=f"psb{m}")
                                       for m in range(NK)]
                    return ps_tiles.get(b)

                def emit_proj(b, m):
                    """Input-projection matmuls for block b, output half m."""
                    if b >= nblocks:
                        return
                    t0 = PBLK * b
                    ps = ps_tiles[b]
                    for d in range(4):
                        for k in range(NK):
                            if d in (0, 2):
                                nc.tensor.matmul(
                                    ps[m][:, d, :],
                                    wi[k][:, d, m, :],
                                    proj_rhs_blk(k, d, t0),
                                    start=(k == 0 and d in (0, 2)),
                                    stop=False, skip_group_check=True)
                            else:
                                for ti in range(PBLK):
                                    nc.tensor.matmul(
                                        ps[m][:, d, ti * S:(ti + 1) * S],
                                        wi[k][:, d, m, :],
                                        proj_rhs_step(k, d, t0 + ti),
                                        start=False,
                                        stop=False, skip_group_check=True)
                        if use_kbias:
                            nc.tensor.matmul(
                                ps[m][:, d, :],
                                kb[:, d * C + m * 128: d * C + (m + 1) * 128],
                                ones_row[:],
                                start=False, stop=False, skip_group_check=True)

                # prefetch ALL per-step gate rows up front; the pool's
                # slot reuse (bufs) self-paces the DMAs against consumers
                gb_tiles = {}
                for tt in range(1, S):
                    g_t = gbp.tile([128, 4, S], DBF, tag="gb", name="gb")
                    nc.sync.dma_start(
                        g_t[:], _dram_ap(gd, tt * 4 * S,
                                         [[0, 128], [1, 4 * S]]))
                    gb_tiles[tt] = g_t

                alloc_ps(0)
                for m in range(NK):
                    emit_proj(0, m)
                for b in range(nblocks):
                    t0 = PBLK * b
                    ps = ps_tiles.pop(b)
                    alloc_ps(b + 1)
                    for ti in range(PBLK):
                        t = t0 + ti
                        slot = t % DEPTH
                        # --- state matmuls (skip t=0), m-major: ps[0] is
                        # complete after the first half, so the m0 gate op
                        # overlaps the m1 matmuls; next step's k-consumers
                        # of hg[0] come first so hg[1] can still be in flight ---
                        if t + 1 < S:
                            gb = gb_tiles.pop(t + 1)
                            hg_nxt = [hgp.tile([128, 4, S], DBF, tag=f"hg{m}",
                                               name=f"hg{m}")
                                      for m in range(NK)]
                        else:
                            gb = None
                        for m in range(NK):
                            if t > 0:
                                for k in range(NK):
                                    for d in range(4):
                                        nc.tensor.matmul(
                                            ps[m][:, d, ti * S:(ti + 1) * S],
                                            ws[k][:, d, m, :],
                                            hg_cur[k][:, d, :],
                                            start=False,
                                            stop=(ti == PBLK - 1 and k == NK - 1
                                                  and d % 2 == 1),
                                            skip_group_check=True)
                            # gate op for this half fires as soon as its own
                            # matmuls are done (boosted: heads the DVE queue)
                            if gb is not None:
                                with tc.high_priority(offset=int(os.environ.get("K_STTP", "0"))):
                                    nc.vector.scalar_tensor_tensor(
                                        hg_nxt[m][:], ps[m][:, :, ti * S:(ti + 1) * S],
                                        0.0, gb[:], OP.max, OP.mult)
                        if gb is not None:
                            hg_cur = hg_nxt
                        # next block's proj fills the PE gap during the gate op
                        emit_proj(b + 1, ti)
                        # --- history evac: h = relu(psum), both block steps in
                        # one ACT op per m-half ---
                        if ti == PBLK - 1:
                            for m in range(NK):
                                nc.scalar.activation(
                                    hist[m][:, :, slot - 1:slot + 1, :],
                                    ps[m][:, :, :], AF.Relu)
                        # --- acc block adds every ABLK steps (gpsimd;
                        # deprioritized so the gate op wins the queues) ---
                        if t % ABLK == ABLK - 1:
                            tb0 = t - (ABLK - 1)
                            s0 = tb0 % DEPTH
                            ctx_p = tc.high_priority(offset=-int(os.environ.get('K_PRIO', '128')))
                            ctx_p.__enter__()
                            for k in range(NK):
                                # lr: cols tb0..t (h outer, w inner)
                                nc.gpsimd.tensor_tensor(
                                    _mkap(acc[k], tb0, [[S, S], [1, ABLK]]),
                                    _mkap(acc[k], tb0, [[S, S], [1, ABLK]]),
                                    _mkap(hist[k], (0 * DEPTH + s0) * S,
                                          [[1, S], [S, ABLK]]),
                                    OP.add)
                                # rl: cols S-1-tb0 down
                                nc.vector.tensor_tensor(
                                    _mkap(acc[k], S - 1 - tb0, [[S, S], [-1, ABLK]]),
                                    _mkap(acc[k], S - 1 - tb0, [[S, S], [-1, ABLK]]),
                                    _mkap(hist[k], (1 * DEPTH + s0) * S,
                                          [[1, S], [S, ABLK]]),
                                    OP.add)
                                # tb: rows tb0..t (slot outer, w inner)
                                nc.gpsimd.tensor_tensor(
                                    _mkap(acc[k], tb0 * S, [[S, ABLK], [1, S]]),
                                    _mkap(acc[k], tb0 * S, [[S, ABLK], [1, S]]),
                                    hist[k][:, 2, s0:s0 + ABLK, :],
                                    OP.add)
                                # bt: rows S-1-tb0 down
                                nc.vector.tensor_tensor(
                                    _mkap(acc[k], (S - 1 - tb0) * S, [[-S, ABLK], [1, S]]),
                                    _mkap(acc[k], (S - 1 - tb0) * S, [[-S, ABLK], [1, S]]),
                                    hist[k][:, 3, s0:s0 + ABLK, :],
                                    OP.add)
                            ctx_p.__exit__(None, None, None)

            # ================= OUTPUT =================
            inv_n = 1.0 / (8.0 * HW)
            with (
                tc.tile_pool(name="ops", bufs=3, space="PSUM") as ops_pool,
                tc.tile_pool(name="oz", bufs=2) as ozp,
                tc.tile_pool(name="ost", bufs=1) as ostp,
                tc.tile_pool(name="obuf", bufs=int(os.environ.get("K_OB", "4"))) as obp,
            ):
                # ---- pass 1: matmuls + z (bf16, in-place over x) + stats ----
                for j in range(NCH):
                    for m in range(NK):
                        pso = ops_pool.tile([128, CH], F32, tag=f"pso{m}")
                        for k in range(NK):
                            nc.tensor.matmul(pso[:], wo[k][:, m, :],
                                             acc[k][:, j * CH:(j + 1) * CH],
                                             start=(k == 0), stop=(k == NK - 1))
                        zap = x[m][:, j * CH:(j + 1) * CH]
                        nc.vector.scalar_tensor_tensor(
                            zap, pso[:], ob[m][:, 0:1], zap,
                            OP.add, OP.add, accum_out=sums[m][:, j:j + 1])
                        N_SQ_DVE = int(os.environ.get("K_SQ_DVE", "0"))
                        if (j * NK + m) % 64 < N_SQ_DVE:
                            junk = ozp.tile([128, CH], DBF, tag="junkb")
                            nc.vector.tensor_tensor_reduce(
                                junk[:], zap, zap, 1.0, 0.0, OP.mult, OP.add,
                                accum_out=sumsq[m][:, j:j + 1])
                        else:
                            junk = ozp.tile([128, CH], F32, tag="junk")
                            nc.scalar.activation(junk[:], zap, AF.Square,
                                                 accum_out=sumsq[m][:, j:j + 1])
                # ---- group stats ----
                ssq = [ostp.tile([128, 2], F32, tag=f"ssq{k}", name=f"ssq{k}") for k in range(NK)]
                for k in range(NK):
                    nc.vector.tensor_reduce(ssq[k][:, 0:1], sums[k][:, 0:NCH],
                                            mybir.AxisListType.X, OP.add)
                    nc.vector.tensor_reduce(ssq[k][:, 1:2], sumsq[k][:, 0:NCH],
                                            mybir.AxisListType.X, OP.add)
                with tc.tile_pool(name="stps", bufs=1, space="PSUM") as stps:
                    psg = stps.tile([GN_GROUPS, 2], F32, tag="psg")
                    for k in range(NK):
                        nc.tensor.matmul(psg[:], gsel[k][:], ssq[k][:],
                                         start=(k == 0), stop=(k == NK - 1))
                    mv = ostp.tile([GN_GROUPS, 2], F32, tag="mv")
                    nc.vector.tensor_scalar(mv[:], psg[:], inv_n, None, OP.mult)
                    mu2 = ostp.tile([GN_GROUPS, 1], F32, tag="mu2")
                    nc.vector.tensor_tensor(mu2[:], mv[:, 0:1], mv[:, 0:1], OP.mult)
                    var = ostp.tile([GN_GROUPS, 1], F32, tag="var")
                    nc.vector.tensor_tensor(var[:], mv[:, 1:2], mu2[:], OP.subtract)
                    sd = ostp.tile([GN_GROUPS, 1], F32, tag="sd")
                    nc.scalar.activation(sd[:], var[:], AF.Sqrt, bias=epsb[:, 0:1])
                    rstd = ostp.tile([GN_GROUPS, 1], F32, tag="rstd")
                    nc.vector.reciprocal(rstd[:], sd[:])
                    mr = ostp.tile([GN_GROUPS, 2], F32, tag="mr")
                    nc.vector.tensor_copy(mr[:, 0:1], mv[:, 0:1])
                    nc.vector.tensor_copy(mr[:, 1:2], rstd[:])
                    # expand group stats to channels; fold gn affine
                    scale = [ostp.tile([128, 1], F32, tag=f"scale{k}", name=f"scale{k}") for k in range(NK)]
                    bias = [ostp.tile([128, 1], F32, tag=f"bias{k}", name=f"bias{k}") for k in range(NK)]
                    pse = stps.tile([128, 2, 2], F32, tag="pse")
                    for k in range(NK):
                        nc.tensor.matmul(pse[:, k, :], gexp[k][:], mr[:],
                                         start=(k == 0), stop=(k == NK - 1),
                                         skip_group_check=True)
                    for k in range(NK):
                        muc = ostp.tile([128, 1], F32, tag=f"muc{k}")
                        rc = ostp.tile([128, 1], F32, tag=f"rc{k}")
                        nc.vector.tensor_copy(muc[:], pse[:, k, 0:1])
                        nc.vector.tensor_copy(rc[:], pse[:, k, 1:2])
                        nc.vector.tensor_tensor(scale[k][:], rc[:], gnw[k][:], OP.mult)
                        tmp = ostp.tile([128, 1], F32, tag=f"tmp{k}")
                        nc.vector.tensor_tensor(tmp[:], muc[:], scale[k][:], OP.mult)
                        nc.vector.tensor_tensor(bias[k][:], gnb[k][:], tmp[:], OP.subtract)

                    # ---- pass 2: affine on stored z; 4-chunk staging
                    # tiles so each out DMA covers 2048 positions (fewer
                    # HWDGE descriptor-gen slots) ----
                    N_AF_POOL = int(os.environ.get("K_AF_POOL", "0"))
                    N_AF_ACT = int(os.environ.get("K_AF_ACT", "0"))
                    OB = 4
                    u = 0
                    for jb in range(0, NCH, OB):
                        for m in range(NK):
                            of = obp.tile([128, OB * CH], DBF, tag="of",
                                          name="of")
                            for q in range(OB):
                                j = jb + q
                                idx = u % 64
                                u += 1
                                zap = x[m][:, j * CH:(j + 1) * CH]
                                oq = of[:, q * CH:(q + 1) * CH]
                                if idx < N_AF_POOL:
                                    nc.gpsimd.tensor_scalar(
                                        oq, zap, scale[m][:, 0:1],
                                        bias[m][:, 0:1], OP.mult, OP.add)
                                elif idx < N_AF_POOL + N_AF_ACT:
                                    nc.scalar.activation(
                                        oq, zap, AF.Identity,
                                        bias=bias[m][:, 0:1],
                                        scale=scale[m][:, 0:1])
                                else:
                                    nc.vector.tensor_scalar(
                                        oq, zap, scale[m][:, 0:1],
                                        bias[m][:, 0:1], OP.mult, OP.add)
                            nc.sync.dma_start(
                                _dram_ap(out_d, m * 128 * HW + jb * CH,
                                         [[HW, 128], [1, OB * CH]]),
                                of[:])
    nc.compile()
    return nc


_CACHE = {}


def _get_program(beta, use_kbias):
    key = (float(beta), bool(use_kbias))
    if key not in _CACHE:
        _CACHE[key] = build_program(beta, use_kbias)
    return _CACHE[key]


def make_host_inputs(feature, boundary_logits, beta, W_in, b_in, W_s, b_s,
                     p_bias, out_w, out_b, gn_w, gn_b):
    wi_t = np.ascontiguousarray(
        np.transpose(np.asarray(W_in, np.float32), (0, 2, 1))).astype(BF)
    ws_t = np.ascontiguousarray(
        np.transpose(np.asarray(W_s, np.float32), (0, 2, 1))).astype(BF)
    wo_t = np.ascontiguousarray(np.asarray(out_w, np.float32).T).astype(BF)
    kbv = (np.asarray(b_in, np.float32) + np.asarray(b_s, np.float32)
           + np.asarray(p_bias, np.float32))
    use_kbias = bool(np.any(kbv != 0.0))
    cpg = C // GN_GROUPS
    gsel = np.zeros((2, 128, GN_GROUPS), np.float32)
    gexp = np.zeros((2, GN_GROUPS, 128), np.float32)
    for k in range(2):
        for p in range(128):
            g = (k * 128 + p) // cpg
            gsel[k, p, g] = 1.0
            gexp[k, g, p] = 1.0
    common = {
        "wi_t": wi_t, "ws_t": ws_t, "wo_t": wo_t,
        "ob": np.asarray(out_b, np.float32),
        "gnw": np.asarray(gn_w, np.float32),
        "gnb": np.asarray(gn_b, np.float32),
        "gsel": gsel, "gexp": gexp,
        "ident": np.eye(128, dtype=np.float32),
        "revid": np.eye(128, dtype=np.float32)[::-1].copy(),
    }
    if use_kbias:
        common["kb"] = kbv.astype(BF)
    B = np.asarray(feature).shape[0]
    in_maps = []
    for b in range(B):
        m = dict(common)
        m["xb"] = np.asarray(feature[b], np.float32).astype(BF)
        m["bl"] = np.asarray(boundary_logits[b], np.float32).reshape(S, S)
        in_maps.append(m)
    return in_maps, float(np.asarray(beta).reshape(-1)[0]), use_kbias


def kernel(feature, boundary_logits, beta, W_in, b_in, W_s, b_s, p_bias,
           out_w, out_b, gn_w, gn_b):
    feature = np.asarray(feature)
    B = feature.shape[0]
    in_maps, beta_v, use_kbias = make_host_inputs(
        feature, boundary_logits, beta, W_in, b_in, W_s, b_s, p_bias,
        out_w, out_b, gn_w, gn_b)
    nc = _get_program(beta_v, use_kbias)
    res = run_bass_kernel_spmd(nc, in_maps, core_ids=list(range(B)))
    out = np.stack([np.asarray(r["out"]) for r in res.results], axis=0)
    return out.astype(np.float32)

